# revision 14
# baseline (speedup 1.0000x reference)
"""DGCNN segmentation forward pass on 8 Trainium2 NeuronCores.

Sharding: core c handles half h=c%2 of cloud b=c//2 (2048 points each).
Within a pair {2b, 2b+1}: AllGather of per-layer features (KNN and the
neighbor gathers need the full cloud), AllReduce-max for the global
embedding. Everything else is local. Rank order == global row order, so
the AllGather output is the canonical full cloud on both cores.

EdgeConv factorization (BN folded into conv weights on the host):
  max_k relu(bn(W·[x_j - x_i; x_i]))
  = relu( maxpool_{j in knn(i)} (x_j @ An^T) + x_i @ Bw^T + bias )
with An = s*Wn, Bw = s*(Wc - Wn). Each layer is then: one dense matmul
over all points (A'), a KNN top-20, a 20-row gather+maxpool of A', and a
per-point matmul (B'').

KNN: PE computes psum = x_i·x_j - sq_j/2 (ones-row fused into the
matmul), ACT emits negd = 2*psum - sq_i = -(d^2). Top-24 of negd per row
via DVE max8/max_index/match_replace; slot 0 is always self
(negd[i,i] ~ 0 is the row max), slots 1..20 are the 20 nearest
neighbors. top_k tie-break (lowest index) matches the reference.
"""
import sys
sys.path.insert(0, '/opt/trn_rl_repo')

import numpy as np

import concourse.bass as bass
import concourse.mybir as mybir
import concourse.tile as tile
from concourse import bacc
from concourse.bass_utils import run_bass_kernel_spmd

B, N, K, NCLS, EMB = 4, 4096, 20, 50, 1024
HALF = N // 2
NT_OWN = HALF // 128     # 16
NT_FULL = N // 128       # 32
NCORES = 8
EPS = 1e-5
F32 = mybir.dt.float32
U32 = mybir.dt.uint32
AF = mybir.ActivationFunctionType
ALU = mybir.AluOpType
GROUPS = [[0, 1], [2, 3], [4, 5], [6, 7]]
NEG_BIG = -1e30

C_IN = [3, 64, 64, 128]
C_OUT = [64, 64, 128, 256]

_CACHE = {}
DEBUG_IDX = False


def _max_over_k_inplace(nc, g, c):
    """In-place pairwise max over the K=20 dim of g [128, 20, c].
    Result lands in g[:, 0, :]. Writes always trail reads elementwise."""
    mm = lambda o, a, b: nc.vector.tensor_tensor(out=o, in0=a, in1=b, op=ALU.max)
    mm(g[:, 0:10, :], g[:, 0:20:2, :], g[:, 1:20:2, :])   # 20 -> 10
    mm(g[:, 0:5, :], g[:, 0:10:2, :], g[:, 1:10:2, :])    # 10 -> 5
    mm(g[:, 0:2, :], g[:, 0:4:2, :], g[:, 1:4:2, :])      # 4  -> 2
    mm(g[:, 0:1, :], g[:, 0:1, :], g[:, 1:2, :])          # 2  -> 1
    mm(g[:, 0:1, :], g[:, 0:1, :], g[:, 4:5, :])          # + leftover 5th
    return g[:, 0, :]


def build_program():
    nc = bacc.Bacc("TRN2", target_bir_lowering=False, debug=False,
                   num_devices=NCORES)

    def din(name, shape, dt=F32):
        return nc.dram_tensor(name, shape, dt, kind="ExternalInput")

    io = {}
    io["l1_lhsT"] = din("l1_lhsT", [4, HALF])    # [xyzT_own; ones]
    io["l1_rhs"] = din("l1_rhs", [4, N])         # [xyzT_full; -sq/2]
    io["nsq1"] = din("nsq1", [128, NT_OWN])      # -sq_own, col t = tile t
    F16 = mybir.dt.float16
    # layer-4 conv + classifier weights never feed a later KNN, so fp16
    # shipping (cast to fp32 on load) costs ~2e-4 relative, saves ~16MB
    # of tunnel transfer per call.
    for l in range(4):
        io[f"aw{l}"] = din(f"aw{l}", [C_IN[l], C_OUT[l]],
                           F16 if l == 3 else F32)
    io["b1rhs"] = din("b1rhs", [4, C_OUT[0]])    # [BwT1; bias1]
    for l in range(1, 4):
        io[f"bw{l}"] = din(f"bw{l}", [C_IN[l], C_OUT[l]],
                           F16 if l == 3 else F32)
        io[f"bb{l}"] = din(f"bb{l}", [1, C_OUT[l]])
    gkdims = [64, 64, 128, 128, 128]
    for i, d in enumerate(gkdims):
        io[f"gk{i}"] = din(f"gk{i}", [d, EMB], F16)
    io["gbias"] = din("gbias", [1, EMB])
    io["wgc"] = din("wgc", [EMB, 256], F16)
    for i, d in enumerate(gkdims):
        io[f"cx{i}"] = din(f"cx{i}", [d, 256], F16)
    io["c1b"] = din("c1b", [1, 256])
    io["c2wa"] = din("c2wa", [128, 256], F16)
    io["c2wb"] = din("c2wb", [128, 256], F16)
    io["c2b"] = din("c2b", [1, 256])
    io["clwa"] = din("clwa", [128, NCLS], F16)
    io["clwb"] = din("clwb", [128, NCLS], F16)
    io["clb"] = din("clb", [1, NCLS])

    io["logits"] = nc.dram_tensor("logits", [HALF, NCLS], F32,
                                  kind="ExternalOutput")
    if DEBUG_IDX:
        for l in range(4):
            io[f"idx{l + 1}"] = nc.dram_tensor(f"idx{l + 1}", [HALF, 24], U32,
                                               kind="ExternalOutput")

    with tile.TileContext(nc) as tc:
        _body(nc, tc, io)
    nc.compile()
    return nc


def _body(nc, tc, io):
    from contextlib import ExitStack
    with ExitStack() as ctx:
        sb = ctx.enter_context(tc.tile_pool(name="sb", bufs=2))
        persist = ctx.enter_context(tc.tile_pool(name="persist", bufs=1))
        ring = ctx.enter_context(tc.tile_pool(name="ring", bufs=1))
        negd_p = ctx.enter_context(tc.tile_pool(name="negd", bufs=2))
        gp = ctx.enter_context(tc.tile_pool(name="gather", bufs=1))
        psD = ctx.enter_context(tc.tile_pool(name="psD", bufs=2, space="PSUM"))
        psM = ctx.enter_context(tc.tile_pool(name="psM", bufs=3, space="PSUM"))
        psG = ctx.enter_context(tc.tile_pool(name="psG", bufs=1, space="PSUM"))
        dram = ctx.enter_context(tc.tile_pool(name="dram", bufs=1, space="DRAM"))

        def wload(dst_ap, src_handle):
            eng = nc.gpsimd if src_handle.dtype != F32 else nc.sync
            eng.dma_start(dst_ap, src_handle[:])

        from concourse.masks import make_identity
        ident = persist.tile([128, 128], F32, tag="ident")
        make_identity(nc, ident[:])
        # one [1,128] ones row reused as the K=1 lhsT for every tile
        ones_r = persist.tile([1, 128], F32, tag="ones")
        nc.vector.memset(ones_r[:], 1.0)

        # layer-1 tensors from host (l1_rhs shares the xTf ring slot,
        # l1_lhsT shares the x2T slot: dead before x2T is written)
        l1rhs_sb = ring.tile([4, N], F32, tag="xTf")
        nc.sync.dma_start(l1rhs_sb[:], io["l1_rhs"][:])
        l1lhs_sb = persist.tile([4, HALF], F32, tag="l1x2")
        nc.sync.dma_start(l1lhs_sb[:], io["l1_lhsT"][:])
        nsq1_sb = persist.tile([128, NT_OWN], F32, tag="nsq1")
        nc.sync.dma_start(nsq1_sb[:], io["nsq1"][:])

        xT_own = l1lhs_sb          # [4, HALF] with ones row fused (layer 1)
        xT_full = l1rhs_sb         # [4, N] with -sq/2 row fused (layer 1)
        nsqh_row = None
        nsq_own = nsq1_sb

        xT_cls = [None] * 5        # x1T, x2T, x3T, x4Ta, x4Tb

        for l in range(4):
            cin, cout = C_IN[l], C_OUT[l]
            fused = (l == 0)

            # destination(s) for this layer's own-feature transposes
            if l == 0:
                x1T = persist.tile([65, HALF], F32, tag="x1T")
                nc.vector.memset(x1T[64:65, :], 1.0)
                dsts = [x1T]
                xT_cls[0] = x1T
            elif l == 1:
                x2T = persist.tile([65, HALF], F32, tag="l1x2")
                nc.vector.memset(x2T[64:65, :], 1.0)
                dsts = [x2T]
                xT_cls[1] = x2T
            elif l == 2:
                x3T = persist.tile([128, HALF], F32, tag="x3T")
                dsts = [x3T]
                xT_cls[2] = x3T
            else:
                x4Ta = persist.tile([128, HALF], F32, tag="x4Ta")
                x4Tb = persist.tile([128, HALF], F32, tag="x4Tb")
                dsts = [x4Ta, x4Tb]
                xT_cls[3], xT_cls[4] = x4Ta, x4Tb

            # ---- A' = x_full @ An^T -> DRAM [N, cout]
            a_rhs = persist.tile([cin, cout], F32, tag="awsb")
            wload(a_rhs[:], io[f"aw{l}"])
            A_dram = dram.tile([N, cout], F32, tag=f"Adram{l}")
            for j in range(NT_FULL):
                jsl = slice(j * 128, (j + 1) * 128)
                pa = psM.tile([128, cout], F32, tag="psm")
                nc.tensor.matmul(pa[:], xT_full[0:cin, jsl], a_rhs[:],
                                 start=True, stop=True)
                asb = sb.tile([128, cout], F32, tag="asb")
                nc.scalar.copy(out=asb[:], in_=pa[:])
                nc.sync.dma_start(A_dram[jsl, :], asb[:])

            # ---- B'' weights
            if fused:
                b_rhs = persist.tile([4, cout], F32, tag="bwsb")
                nc.sync.dma_start(b_rhs[:], io["b1rhs"][:])
                bb_sb = None
            elif l < 3:
                # bias folded as row cin (pairs with the xT ones row)
                b_rhs = persist.tile([cin + 1, cout], F32, tag="bwsb")
                wload(b_rhs[0:cin, :], io[f"bw{l}"])
                nc.sync.dma_start(b_rhs[cin:cin + 1, :], io[f"bb{l}"][:])
                bb_sb = None
            else:
                b_rhs = persist.tile([cin, cout], F32, tag="bwsb")
                wload(b_rhs[:], io[f"bw{l}"])
                bb_sb = persist.tile([1, cout], F32, tag="bbsb")
                nc.sync.dma_start(bb_sb[:], io[f"bb{l}"][:])

            if l < 3:
                own_bounce = dram.tile([HALF, cout], F32, tag=f"ownb{l}")
                nsq_own_next = persist.tile([128, NT_OWN], F32, tag=f"nsqo{l % 2}")

            # l=0: host fused [xyzT;ones]x[xyzT;-sq/2]; l=1,2: device-fused
            # ones/-sq rows (kdim=65); l=3: separate ones x (-sq/2) matmul
            kdim = 4 if fused else (cin + 1 if l < 3 else cin)
            allfused = l < 3

            for t in range(NT_OWN):
                tsl = slice(t * 128, (t + 1) * 128)
                negd = negd_p.tile([128, N], F32, tag="negd")
                for ch in range(4):
                    csl = slice(ch * 1024, (ch + 1) * 1024)
                    pd = psD.tile([128, 1024], F32, tag="psd")
                    for sub in range(2):
                        ssl = slice(ch * 1024 + sub * 512,
                                    ch * 1024 + (sub + 1) * 512)
                        psl = slice(sub * 512, (sub + 1) * 512)
                        nc.tensor.matmul(pd[:, psl], xT_own[0:kdim, tsl],
                                         xT_full[0:kdim, ssl],
                                         start=True, stop=allfused)
                        if not allfused:
                            nc.tensor.matmul(pd[:, psl], ones_r[:],
                                             nsqh_row[:, ssl],
                                             start=False, stop=True)
                    nc.scalar.activation(negd[:, csl], pd[:], AF.Identity,
                                         bias=nsq_own[:, t:t + 1], scale=2.0)

                # top-24 (slot 0 = self)
                idx = sb.tile([128, 24], U32, tag="tkidx")
                vals = sb.tile([128, 24], F32, tag="tkvals")
                for r in range(3):
                    rsl = slice(r * 8, (r + 1) * 8)
                    nc.vector.max(out=vals[:, rsl], in_=negd[:])
                    nc.vector.max_index(out=idx[:, rsl], in_max=vals[:, rsl],
                                        in_values=negd[:])
                    if r < 2:
                        nc.vector.match_replace(out=negd[:],
                                                in_to_replace=vals[:, rsl],
                                                in_values=negd[:],
                                                imm_value=NEG_BIG)
                if DEBUG_IDX:
                    nc.sync.dma_start(io[f"idx{l + 1}"][tsl, :], idx[:])

                # gather 20 neighbor rows of A' and max-pool them
                g = gp.tile([128, K, cout], F32, tag="gbuf")
                for r in range(K):
                    nc.gpsimd.indirect_dma_start(
                        out=g[:, r, :], out_offset=None, in_=A_dram[:],
                        in_offset=bass.IndirectOffsetOnAxis(
                            ap=idx[:, 1 + r:2 + r], axis=0))
                m_ap = _max_over_k_inplace(nc, g, cout)

                # B'' + relu
                pb = psM.tile([128, cout], F32, tag="psm")
                nc.tensor.matmul(pb[:], xT_own[0:kdim, tsl], b_rhs[:],
                                 start=True, stop=allfused)
                if not allfused:
                    nc.tensor.matmul(pb[:], ones_r[:], bb_sb[:],
                                     start=False, stop=True)
                xt = sb.tile([128, cout], F32, tag="xout")
                nc.vector.tensor_tensor(out=xt[:], in0=m_ap, in1=pb[:],
                                        op=ALU.add)
                nc.scalar.activation(xt[:], xt[:], AF.Relu)

                if l < 3:
                    # -sq_own for next layer's distance bias
                    scr = sb.tile([128, cout], F32, tag="sqscr")
                    sqc = sb.tile([128, 1], F32, tag="sqcol")
                    nc.scalar.activation(scr[:], xt[:], AF.Square,
                                         accum_out=sqc[:])
                    nc.scalar.activation(nsq_own_next[:, t:t + 1], sqc[:],
                                         AF.Copy, scale=-1.0)
                    nc.sync.dma_start(own_bounce[tsl, :], xt[:])

                # transpose own tile into the persistent xT chunks
                for cb, dst in enumerate(dsts):
                    rows = min(128, cout - 128 * cb)
                    pt = psM.tile([128, 128], F32, tag="psm")
                    nc.tensor.transpose(pt[:rows, :],
                                        xt[:, cb * 128:cb * 128 + rows],
                                        ident[:])
                    nc.scalar.copy(out=dst[:rows, tsl], in_=pt[:rows, :])

            # ---- between layers: AllGather + rebuild full-side state
            if l < 3:
                full_bounce = dram.tile([N, cout], F32, tag=f"fullb{l}")
                nc.gpsimd.collective_compute(
                    "AllGather", ALU.bypass, replica_groups=GROUPS,
                    ins=[own_bounce.opt()], outs=[full_bounce.opt()])

                cn = C_IN[l + 1]
                # cin=64 next layers get the -sq/2 row fused at row 64
                xT_full_next = ring.tile([cn + 1 if cn == 64 else cn, N],
                                         F32, tag="xTf")
                if cn == 64:
                    dest_row = xT_full_next[64:65, :]
                    nsqh_next = None
                else:
                    nsqh_next = persist.tile([1, N], F32, tag="nsqhr")
                    dest_row = nsqh_next[0:1, :]
                for j in range(NT_FULL):
                    jsl = slice(j * 128, (j + 1) * 128)
                    xf = sb.tile([128, cout], F32, tag="xfull")
                    nc.sync.dma_start(xf[:], full_bounce[jsl, :])
                    pt = psM.tile([128, 128], F32, tag="psm")
                    nc.tensor.transpose(pt[:cout, :], xf[:], ident[:])
                    nc.scalar.copy(out=xT_full_next[0:cout, jsl],
                                   in_=pt[:cout, :])
                    scr = sb.tile([128, cout], F32, tag="sqscr")
                    sqc = sb.tile([128, 1], F32, tag="sqcol")
                    nc.scalar.activation(scr[:], xf[:], AF.Square,
                                         accum_out=sqc[:])
                    sqh = sb.tile([128, 1], F32, tag="sqh")
                    nc.scalar.activation(sqh[:], sqc[:], AF.Copy, scale=-0.5)
                    nc.sync.dma_start(dest_row[0:1, jsl], sqh[:])

                xT_own = dsts[0][:]
                xT_full = xT_full_next
                nsqh_row = nsqh_next
                nsq_own = nsq_own_next

        x1T, x2T, x3T, x4Ta, x4Tb = xT_cls

        # ============ classifier pass A: g = relu(x_cat@Wg^T+b), gmax ============
        g_acc = persist.tile([128, EMB], F32, tag="gacc")
        for hh in range(2):
            nsl = slice(hh * 512, (hh + 1) * 512)
            gw = []
            for i, nm in enumerate(["gk0", "gk1", "gk2", "gk3", "gk4"]):
                gwt = sb.tile([list(io[nm].shape)[0], 512], F32, tag=f"gkh{i}",
                              bufs=1)
                eng = nc.gpsimd if io[nm].dtype != F32 else nc.sync
                eng.dma_start(gwt[:], io[nm][:, nsl])
                gw.append(gwt)
            gbt = sb.tile([1, 512], F32, tag="gbh", bufs=1)
            nc.sync.dma_start(gbt[:], io["gbias"][:, nsl])
            lhss = [x1T[0:64, :], x2T[0:64, :], x3T[:], x4Ta[:], x4Tb[:]]
            chunks = list(zip(lhss, gw))
            for t in range(NT_OWN):
                tsl = slice(t * 128, (t + 1) * 128)
                pg = psG.tile([128, 512], F32, tag="psg")
                for ci, (lhs_t, w_t) in enumerate(chunks):
                    nc.tensor.matmul(pg[:], lhs_t[:, tsl], w_t[:],
                                     start=(ci == 0), stop=False)
                nc.tensor.matmul(pg[:], ones_r[:], gbt[:],
                                 start=False, stop=True)
                gt = sb.tile([128, 512], F32, tag="gtile")
                nc.scalar.activation(gt[:], pg[:], AF.Relu)
                if t == 0:
                    nc.vector.tensor_copy(g_acc[:, nsl], gt[:])
                else:
                    nc.vector.tensor_tensor(out=g_acc[:, nsl],
                                            in0=g_acc[:, nsl],
                                            in1=gt[:], op=ALU.max)

        # gmax over points: transpose blocks + pool, then AllReduce(max)
        gmax8 = persist.tile([128, 8], F32, tag="gmax8")
        for b in range(8):
            pt = psM.tile([128, 128], F32, tag="psm")
            nc.tensor.transpose(pt[:], g_acc[:, b * 128:(b + 1) * 128], ident[:])
            nc.vector.pool_max(out=gmax8[:, b:b + 1], in_=pt[:])
        gmax_b = dram.tile([128, 8], F32, tag="gmaxb")
        gmax_rb = dram.tile([128, 8], F32, tag="gmaxrb")
        nc.sync.dma_start(gmax_b[:], gmax8[:])
        nc.gpsimd.collective_compute("AllReduce", ALU.max, replica_groups=GROUPS,
                                     ins=[gmax_b.opt()], outs=[gmax_rb.opt()])
        gmaxr = persist.tile([128, 8], F32, tag="gmaxr")
        nc.sync.dma_start(gmaxr[:], gmax_rb[:])

        # gv = Wgc' @ gmax + c1 bias -> [1, 256]
        c1b_sb = persist.tile([1, 256], F32, tag="c1b")
        nc.sync.dma_start(c1b_sb[:], io["c1b"][:])
        pgv = psM.tile([1, 256], F32, tag="psm")
        for b in range(8):
            wc = sb.tile([128, 256], F32, tag="wgch")
            nc.gpsimd.dma_start(wc[:], io["wgc"][b * 128:(b + 1) * 128, :])
            nc.tensor.matmul(pgv[:], gmaxr[:, b:b + 1], wc[:],
                             start=(b == 0), stop=False)
        nc.tensor.matmul(pgv[:], ones_r[0:1, 0:1], c1b_sb[:],
                         start=False, stop=True)
        gv = persist.tile([1, 256], F32, tag="gv")
        nc.scalar.copy(out=gv[:], in_=pgv[:])

        # ============ classifier pass B: c1 -> c2 -> logits ============
        wsb = {}
        for name in ["cx0", "cx1", "cx2", "cx3", "cx4", "c2wa", "c2wb", "c2b",
                     "clwa", "clwb", "clb"]:
            wt = persist.tile(list(io[name].shape), F32, tag=f"w_{name}")
            wload(wt[:], io[name])
            wsb[name] = wt

        for t in range(NT_OWN):
            tsl = slice(t * 128, (t + 1) * 128)
            p1 = psM.tile([128, 256], F32, tag="psm")
            chunks = [(x1T[0:64, :], "cx0"), (x2T[0:64, :], "cx1"),
                      (x3T[:], "cx2"), (x4Ta[:], "cx3"), (x4Tb[:], "cx4")]
            for ci, (lhs_t, wname) in enumerate(chunks):
                nc.tensor.matmul(p1[:], lhs_t[:, tsl], wsb[wname][:],
                                 start=(ci == 0), stop=False)
            nc.tensor.matmul(p1[:], ones_r[:], gv[:],
                             start=False, stop=True)
            c1t = sb.tile([128, 256], F32, tag="c1t")
            nc.scalar.activation(c1t[:], p1[:], AF.Relu)

            c1Ta = sb.tile([128, 128], F32, tag="c1Ta")
            c1Tb = sb.tile([128, 128], F32, tag="c1Tb")
            for cb, dstt in enumerate([c1Ta, c1Tb]):
                pt = psM.tile([128, 128], F32, tag="psm")
                nc.tensor.transpose(pt[:], c1t[:, cb * 128:(cb + 1) * 128],
                                    ident[:])
                nc.scalar.copy(out=dstt[:], in_=pt[:])

            p2 = psM.tile([128, 256], F32, tag="psm")
            nc.tensor.matmul(p2[:], c1Ta[:], wsb["c2wa"][:],
                             start=True, stop=False)
            nc.tensor.matmul(p2[:], c1Tb[:], wsb["c2wb"][:],
                             start=False, stop=False)
            nc.tensor.matmul(p2[:], ones_r[:], wsb["c2b"][:],
                             start=False, stop=True)
            c2t = sb.tile([128, 256], F32, tag="c2t")
            nc.scalar.activation(c2t[:], p2[:], AF.Relu)

            c2Ta = sb.tile([128, 128], F32, tag="c2Ta")
            c2Tb = sb.tile([128, 128], F32, tag="c2Tb")
            for cb, dstt in enumerate([c2Ta, c2Tb]):
                pt = psM.tile([128, 128], F32, tag="psm")
                nc.tensor.transpose(pt[:], c2t[:, cb * 128:(cb + 1) * 128],
                                    ident[:])
                nc.scalar.copy(out=dstt[:], in_=pt[:])

            p3 = psM.tile([128, NCLS], F32, tag="psm")
            nc.tensor.matmul(p3[:], c2Ta[:], wsb["clwa"][:],
                             start=True, stop=False)
            nc.tensor.matmul(p3[:], c2Tb[:], wsb["clwb"][:],
                             start=False, stop=False)
            nc.tensor.matmul(p3[:], ones_r[:], wsb["clb"][:],
                             start=False, stop=True)
            lo = sb.tile([128, NCLS], F32, tag="lot")
            nc.scalar.copy(out=lo[:], in_=p3[:])
            nc.sync.dma_start(io["logits"][tsl, :], lo[:])


# ======================= host side =======================

def _np(x):
    return np.asarray(x, dtype=np.float32)


def _fold_bn(bnp):
    s = _np(bnp["gamma"]) / np.sqrt(_np(bnp["var"]) + EPS)
    b = _np(bnp["beta"]) - _np(bnp["mean"]) * s
    return s, b


def _weight_inputs(params):
    io = {}
    for l in range(4):
        w = _np(params[f"ec{l + 1}_w"])          # [C_out, 2C]
        s, bias = _fold_bn(params[f"ec{l + 1}_bn"])
        C = C_IN[l]
        Wn, Wc = w[:, :C], w[:, C:]
        An = s[:, None] * Wn
        Bw = s[:, None] * (Wc - Wn)
        io[f"aw{l}"] = np.ascontiguousarray(An.T).astype(
            np.float16 if l == 3 else np.float32)
        if l == 0:
            io["b1rhs"] = np.concatenate([Bw.T, bias[None, :]], 0)
        else:
            io[f"bw{l}"] = np.ascontiguousarray(Bw.T).astype(
                np.float16 if l == 3 else np.float32)
            io[f"bb{l}"] = bias[None, :].copy()
    sg, bg = _fold_bn(params["g_bn"])
    WgT = np.ascontiguousarray((sg[:, None] * _np(params["g_w"])).T)  # [512, 1024]
    bounds = [0, 64, 128, 256, 384, 512]
    for i in range(5):
        io[f"gk{i}"] = np.ascontiguousarray(
            WgT[bounds[i]:bounds[i + 1], :]).astype(np.float16)
    io["gbias"] = bg[None, :].copy()
    s1, b1 = _fold_bn(params["c1_bn"])
    W1T = np.ascontiguousarray((s1[:, None] * _np(params["c1_w"])).T)  # [1536, 256]
    for i in range(5):
        io[f"cx{i}"] = np.ascontiguousarray(
            W1T[bounds[i]:bounds[i + 1], :]).astype(np.float16)
    io["wgc"] = np.ascontiguousarray(W1T[512:, :]).astype(np.float16)
    io["c1b"] = b1[None, :].copy()
    s2, b2 = _fold_bn(params["c2_bn"])
    W2T = np.ascontiguousarray((s2[:, None] * _np(params["c2_w"])).T)  # [256, 256]
    io["c2wa"], io["c2wb"] = (W2T[:128].astype(np.float16),
                              W2T[128:].astype(np.float16))
    io["c2b"] = b2[None, :].copy()
    WcT = np.ascontiguousarray(_np(params["cls_w"]).T)                 # [256, 50]
    io["clwa"], io["clwb"] = (WcT[:128].astype(np.float16),
                              WcT[128:].astype(np.float16))
    io["clb"] = _np(params["cls_b"])[None, :].copy()
    return io


def _get_program():
    if "nc" not in _CACHE:
        _CACHE["nc"] = build_program()
    return _CACHE["nc"]


def _get_runner():
    """Build the sharded PJRT callable ONCE; reuse across kernel() calls
    (run_bass_via_pjrt re-traces every call, which costs ~2s wall)."""
    if "runner" in _CACHE:
        return _CACHE["runner"]
    import jax
    from jax.experimental.shard_map import shard_map
    from jax.sharding import Mesh, PartitionSpec
    from concourse import bass2jax

    nc = _get_program()
    bass2jax.install_neuronx_cc_hook()

    partition_name = (nc.partition_id_tensor.name
                      if nc.partition_id_tensor else None)
    in_names, out_names, out_avals, zero_shapes = [], [], [], []
    for alloc in nc.m.functions[0].allocations:
        if not isinstance(alloc, mybir.MemoryLocationSet):
            continue
        name = alloc.memorylocations[0].name
        if alloc.kind == "ExternalInput":
            if name != partition_name:
                in_names.append(name)
        elif alloc.kind == "ExternalOutput":
            shape = tuple(alloc.tensor_shape)
            dtype = mybir.dt.np(alloc.dtype)
            out_names.append(name)
            out_avals.append(jax.core.ShapedArray(shape, dtype))
            zero_shapes.append((shape, dtype))
    n_params = len(in_names)
    n_outs = len(out_names)
    all_in = in_names + out_names + ([partition_name] if partition_name else [])
    donate = tuple(range(n_params, n_params + n_outs))

    def _bass_body(*args):
        operands = list(args)
        if partition_name is not None:
            operands.append(bass2jax.partition_id_tensor())
        outs = bass2jax._bass_exec_p.bind(
            *operands,
            out_avals=tuple(out_avals),
            in_names=tuple(all_in),
            out_names=tuple(out_names),
            lowering_input_output_aliases=(),
            sim_require_finite=True,
            sim_require_nnan=True,
            nc=nc,
        )
        return tuple(outs)

    devices = jax.devices()[:NCORES]
    mesh = Mesh(np.asarray(devices), ("core",))
    # per-core inputs are sharded; identical weights are replicated so the
    # host uploads one copy instead of an 8x concat
    PER_CORE = {"l1_lhsT", "l1_rhs", "nsq1"}
    in_specs = tuple(
        PartitionSpec("core") if nm in PER_CORE else PartitionSpec()
        for nm in in_names) + (PartitionSpec("core"),) * n_outs
    out_specs = (PartitionSpec("core"),) * n_outs
    sharded = jax.jit(
        shard_map(_bass_body, mesh=mesh, in_specs=in_specs,
                  out_specs=out_specs, check_rep=False),
        donate_argnums=donate, keep_unused=True)
    _CACHE["runner"] = (sharded, in_names, out_names, out_avals, zero_shapes,
                        PER_CORE)
    return _CACHE["runner"]


def _run(in_maps):
    import jax
    (sharded, in_names, out_names, out_avals, zero_shapes,
     per_core) = _get_runner()
    concat_in = [
        np.concatenate([np.asarray(in_maps[c][name]) for c in range(NCORES)],
                       axis=0)
        if name in per_core else np.asarray(in_maps[0][name])
        for name in in_names]
    concat_zeros = [np.zeros((NCORES * s[0], *s[1:]), d)
                    for (s, d) in zero_shapes]
    try:
        out_arrs = jax.block_until_ready(sharded(*concat_in, *concat_zeros))
    except Exception:
        concat_zeros = [np.zeros((NCORES * s[0], *s[1:]), d)
                        for (s, d) in zero_shapes]
        out_arrs = jax.block_until_ready(sharded(*concat_in, *concat_zeros))
    results = []
    for c in range(NCORES):
        results.append({
            name: np.asarray(out_arrs[i]).reshape(
                NCORES, *out_avals[i].shape)[c]
            for i, name in enumerate(out_names)})
    return results


def kernel(xyz, params):
    xyz = _np(xyz)                                # [4, 4096, 3]
    wio = _weight_inputs(params)

    in_maps = []
    for core in range(NCORES):
        b, h = core // 2, core % 2
        cloud = xyz[b]                            # [4096, 3]
        own = cloud[h * HALF:(h + 1) * HALF]
        sq_full = (cloud * cloud).sum(-1)         # [4096]
        sq_own = sq_full[h * HALF:(h + 1) * HALF]
        im = dict(wio)
        im["l1_lhsT"] = np.concatenate(
            [own.T, np.ones((1, HALF), np.float32)], 0)
        im["l1_rhs"] = np.concatenate(
            [cloud.T, -0.5 * sq_full[None, :]], 0).astype(np.float32)
        im["nsq1"] = np.ascontiguousarray(
            (-sq_own).reshape(NT_OWN, 128).T)
        in_maps.append(im)

    import time
    t0 = time.perf_counter()
    results = _run(in_maps)
    _CACHE["last_exec_s"] = time.perf_counter() - t0
    out = np.zeros((B, N, NCLS), np.float32)
    for core in range(NCORES):
        b, h = core // 2, core % 2
        out[b, h * HALF:(h + 1) * HALF] = results[core]["logits"]
    _CACHE["last_results"] = results
    return out


# revision 16
# speedup vs baseline: 2.1512x; 2.1512x over previous
"""DGCNN segmentation forward pass on 8 Trainium2 NeuronCores.

Sharding: core c handles half h=c%2 of cloud b=c//2 (2048 points each).
Within a pair {2b, 2b+1}: AllGather of per-layer features (KNN and the
neighbor gathers need the full cloud), AllReduce-max for the global
embedding. Everything else is local. Rank order == global row order, so
the AllGather output is the canonical full cloud on both cores.

EdgeConv factorization (BN folded into conv weights on the host):
  max_k relu(bn(W·[x_j - x_i; x_i]))
  = relu( maxpool_{j in knn(i)} (x_j @ An^T) + x_i @ Bw^T + bias )
with An = s*Wn, Bw = s*(Wc - Wn). Each layer is then: one dense matmul
over all points (A'), a KNN top-20, a 20-row gather+maxpool of A', and a
per-point matmul (B'').

KNN: PE computes psum = x_i·x_j - sq_j/2 (ones-row fused into the
matmul), ACT emits negd = 2*psum - sq_i = -(d^2). Top-24 of negd per row
via DVE max8/max_index/match_replace; slot 0 is always self
(negd[i,i] ~ 0 is the row max), slots 1..20 are the 20 nearest
neighbors. top_k tie-break (lowest index) matches the reference.
"""
import sys
sys.path.insert(0, '/opt/trn_rl_repo')

import numpy as np

import concourse.bass as bass
import concourse.mybir as mybir
import concourse.tile as tile
from concourse import bacc
from concourse.bass_utils import run_bass_kernel_spmd

B, N, K, NCLS, EMB = 4, 4096, 20, 50, 1024
HALF = N // 2
NT_OWN = HALF // 128     # 16
NT_FULL = N // 128       # 32
NCORES = 8
EPS = 1e-5
F32 = mybir.dt.float32
U32 = mybir.dt.uint32
AF = mybir.ActivationFunctionType
ALU = mybir.AluOpType
GROUPS = [[0, 1], [2, 3], [4, 5], [6, 7]]
NEG_BIG = -1e30

C_IN = [3, 64, 64, 128]
C_OUT = [64, 64, 128, 256]

_CACHE = {}
DEBUG_IDX = False


def _max_over_k_inplace(nc, g, c):
    """In-place pairwise max over the K=20 dim of g [128, 20, c].
    Result lands in g[:, 0, :]. Writes always trail reads elementwise."""
    mm = lambda o, a, b: nc.vector.tensor_tensor(out=o, in0=a, in1=b, op=ALU.max)
    mm(g[:, 0:10, :], g[:, 0:20:2, :], g[:, 1:20:2, :])   # 20 -> 10
    mm(g[:, 0:5, :], g[:, 0:10:2, :], g[:, 1:10:2, :])    # 10 -> 5
    mm(g[:, 0:2, :], g[:, 0:4:2, :], g[:, 1:4:2, :])      # 4  -> 2
    mm(g[:, 0:1, :], g[:, 0:1, :], g[:, 1:2, :])          # 2  -> 1
    mm(g[:, 0:1, :], g[:, 0:1, :], g[:, 4:5, :])          # + leftover 5th
    return g[:, 0, :]


def build_program():
    nc = bacc.Bacc("TRN2", target_bir_lowering=False, debug=False,
                   num_devices=NCORES)

    def din(name, shape, dt=F32):
        return nc.dram_tensor(name, shape, dt, kind="ExternalInput")

    io = {}
    io["l1_lhsT"] = din("l1_lhsT", [4, HALF])    # [xyzT_own; ones]
    io["l1_rhs"] = din("l1_rhs", [4, N])         # [xyzT_full; -sq/2]
    io["nsq1"] = din("nsq1", [128, NT_OWN])      # -sq_own, col t = tile t
    F16 = mybir.dt.float16
    # layer-4 conv + classifier weights never feed a later KNN, so fp16
    # shipping (cast to fp32 on load) costs ~2e-4 relative, saves ~16MB
    # of tunnel transfer per call.
    for l in range(4):
        io[f"aw{l}"] = din(f"aw{l}", [C_IN[l], C_OUT[l]],
                           F16 if l == 3 else F32)
    io["b1rhs"] = din("b1rhs", [4, C_OUT[0]])    # [BwT1; bias1]
    for l in range(1, 4):
        io[f"bw{l}"] = din(f"bw{l}", [C_IN[l], C_OUT[l]],
                           F16 if l == 3 else F32)
        io[f"bb{l}"] = din(f"bb{l}", [1, C_OUT[l]])
    gkdims = [64, 64, 128, 128, 128]
    for i, d in enumerate(gkdims):
        io[f"gk{i}"] = din(f"gk{i}", [d, EMB], F16)
    io["gbias"] = din("gbias", [1, EMB])
    io["wgc"] = din("wgc", [EMB, 256], F16)
    for i, d in enumerate(gkdims):
        io[f"cx{i}"] = din(f"cx{i}", [d, 256], F16)
    io["c1b"] = din("c1b", [1, 256])
    io["c2wa"] = din("c2wa", [128, 256], F16)
    io["c2wb"] = din("c2wb", [128, 256], F16)
    io["c2b"] = din("c2b", [1, 256])
    io["clwa"] = din("clwa", [128, NCLS], F16)
    io["clwb"] = din("clwb", [128, NCLS], F16)
    io["clb"] = din("clb", [1, NCLS])

    io["logits"] = nc.dram_tensor("logits", [HALF, NCLS], F32,
                                  kind="ExternalOutput")
    if DEBUG_IDX:
        for l in range(4):
            io[f"idx{l + 1}"] = nc.dram_tensor(f"idx{l + 1}", [HALF, 24], U32,
                                               kind="ExternalOutput")

    with tile.TileContext(nc) as tc:
        _body(nc, tc, io)
    nc.compile()
    return nc


def _body(nc, tc, io):
    from contextlib import ExitStack
    with ExitStack() as ctx:
        sb = ctx.enter_context(tc.tile_pool(name="sb", bufs=2))
        persist = ctx.enter_context(tc.tile_pool(name="persist", bufs=1))
        ring = ctx.enter_context(tc.tile_pool(name="ring", bufs=1))
        negd_p = ctx.enter_context(tc.tile_pool(name="negd", bufs=2))
        gp = ctx.enter_context(tc.tile_pool(name="gather", bufs=1))
        psD = ctx.enter_context(tc.tile_pool(name="psD", bufs=2, space="PSUM"))
        psM = ctx.enter_context(tc.tile_pool(name="psM", bufs=3, space="PSUM"))
        psG = ctx.enter_context(tc.tile_pool(name="psG", bufs=1, space="PSUM"))
        dram = ctx.enter_context(tc.tile_pool(name="dram", bufs=1, space="DRAM"))

        def wload(dst_ap, src_handle):
            eng = nc.gpsimd if src_handle.dtype != F32 else nc.sync
            eng.dma_start(dst_ap, src_handle[:])

        from concourse.masks import make_identity
        ident = persist.tile([128, 128], F32, tag="ident")
        make_identity(nc, ident[:])
        # one [1,128] ones row reused as the K=1 lhsT for every tile
        ones_r = persist.tile([1, 128], F32, tag="ones")
        nc.vector.memset(ones_r[:], 1.0)

        # layer-1 tensors from host (l1_rhs shares the xTf ring slot,
        # l1_lhsT shares the x2T slot: dead before x2T is written)
        l1rhs_sb = ring.tile([4, N], F32, tag="xTf")
        nc.sync.dma_start(l1rhs_sb[:], io["l1_rhs"][:])
        l1lhs_sb = persist.tile([4, HALF], F32, tag="l1x2")
        nc.sync.dma_start(l1lhs_sb[:], io["l1_lhsT"][:])
        nsq1_sb = persist.tile([128, NT_OWN], F32, tag="nsq1")
        nc.sync.dma_start(nsq1_sb[:], io["nsq1"][:])

        xT_own = l1lhs_sb          # [4, HALF] with ones row fused (layer 1)
        xT_full = l1rhs_sb         # [4, N] with -sq/2 row fused (layer 1)
        nsqh_row = None
        nsq_own = nsq1_sb

        xT_cls = [None] * 5        # x1T, x2T, x3T, x4Ta, x4Tb

        for l in range(4):
            cin, cout = C_IN[l], C_OUT[l]
            fused = (l == 0)

            # destination(s) for this layer's own-feature transposes
            if l == 0:
                x1T = persist.tile([65, HALF], F32, tag="x1T")
                nc.vector.memset(x1T[64:65, :], 1.0)
                dsts = [x1T]
                xT_cls[0] = x1T
            elif l == 1:
                x2T = persist.tile([65, HALF], F32, tag="l1x2")
                nc.vector.memset(x2T[64:65, :], 1.0)
                dsts = [x2T]
                xT_cls[1] = x2T
            elif l == 2:
                x3T = persist.tile([128, HALF], F32, tag="x3T")
                dsts = [x3T]
                xT_cls[2] = x3T
            else:
                x4Ta = persist.tile([128, HALF], F32, tag="x4Ta")
                x4Tb = persist.tile([128, HALF], F32, tag="x4Tb")
                dsts = [x4Ta, x4Tb]
                xT_cls[3], xT_cls[4] = x4Ta, x4Tb

            # ---- A' = x_full @ An^T -> DRAM [N, cout]
            a_rhs = persist.tile([cin, cout], F32, tag="awsb")
            wload(a_rhs[:], io[f"aw{l}"])
            A_dram = dram.tile([N, cout], F32, tag=f"Adram{l}")
            for j in range(NT_FULL):
                jsl = slice(j * 128, (j + 1) * 128)
                pa = psM.tile([128, cout], F32, tag="psm")
                nc.tensor.matmul(pa[:], xT_full[0:cin, jsl], a_rhs[:],
                                 start=True, stop=True)
                asb = sb.tile([128, cout], F32, tag="asb")
                nc.scalar.copy(out=asb[:], in_=pa[:])
                nc.sync.dma_start(A_dram[jsl, :], asb[:])

            # ---- B'' weights
            if fused:
                b_rhs = persist.tile([4, cout], F32, tag="bwsb")
                nc.sync.dma_start(b_rhs[:], io["b1rhs"][:])
                bb_sb = None
            elif l < 3:
                # bias folded as row cin (pairs with the xT ones row)
                b_rhs = persist.tile([cin + 1, cout], F32, tag="bwsb")
                wload(b_rhs[0:cin, :], io[f"bw{l}"])
                nc.sync.dma_start(b_rhs[cin:cin + 1, :], io[f"bb{l}"][:])
                bb_sb = None
            else:
                b_rhs = persist.tile([cin, cout], F32, tag="bwsb")
                wload(b_rhs[:], io[f"bw{l}"])
                bb_sb = persist.tile([1, cout], F32, tag="bbsb")
                nc.sync.dma_start(bb_sb[:], io[f"bb{l}"][:])

            if l < 3:
                own_bounce = dram.tile([HALF, cout], F32, tag=f"ownb{l}")
                nsq_own_next = persist.tile([128, NT_OWN], F32, tag=f"nsqo{l % 2}")

            # l=0: host fused [xyzT;ones]x[xyzT;-sq/2]; l=1,2: device-fused
            # ones/-sq rows (kdim=65); l=3: separate ones x (-sq/2) matmul
            kdim = 4 if fused else (cin + 1 if l < 3 else cin)
            allfused = l < 3

            for t in range(NT_OWN):
                tsl = slice(t * 128, (t + 1) * 128)
                negd = negd_p.tile([128, N], F32, tag="negd")
                for ch in range(4):
                    csl = slice(ch * 1024, (ch + 1) * 1024)
                    pd = psD.tile([128, 1024], F32, tag="psd")
                    for sub in range(2):
                        ssl = slice(ch * 1024 + sub * 512,
                                    ch * 1024 + (sub + 1) * 512)
                        psl = slice(sub * 512, (sub + 1) * 512)
                        nc.tensor.matmul(pd[:, psl], xT_own[0:kdim, tsl],
                                         xT_full[0:kdim, ssl],
                                         start=True, stop=allfused)
                        if not allfused:
                            nc.tensor.matmul(pd[:, psl], ones_r[:],
                                             nsqh_row[:, ssl],
                                             start=False, stop=True)
                    nc.scalar.activation(negd[:, csl], pd[:], AF.Identity,
                                         bias=nsq_own[:, t:t + 1], scale=2.0)

                # top-24 (slot 0 = self)
                idx = sb.tile([128, 24], U32, tag="tkidx")
                vals = sb.tile([128, 24], F32, tag="tkvals")
                for r in range(3):
                    rsl = slice(r * 8, (r + 1) * 8)
                    nc.vector.max(out=vals[:, rsl], in_=negd[:])
                    nc.vector.max_index(out=idx[:, rsl], in_max=vals[:, rsl],
                                        in_values=negd[:])
                    if r < 2:
                        nc.vector.match_replace(out=negd[:],
                                                in_to_replace=vals[:, rsl],
                                                in_values=negd[:],
                                                imm_value=NEG_BIG)
                if DEBUG_IDX:
                    nc.sync.dma_start(io[f"idx{l + 1}"][tsl, :], idx[:])

                # gather 20 neighbor rows of A' and max-pool them
                g = gp.tile([128, K, cout], F32, tag="gbuf")
                for r in range(K):
                    nc.gpsimd.indirect_dma_start(
                        out=g[:, r, :], out_offset=None, in_=A_dram[:],
                        in_offset=bass.IndirectOffsetOnAxis(
                            ap=idx[:, 1 + r:2 + r], axis=0))
                m_ap = _max_over_k_inplace(nc, g, cout)

                # B'' + relu
                pb = psM.tile([128, cout], F32, tag="psm")
                nc.tensor.matmul(pb[:], xT_own[0:kdim, tsl], b_rhs[:],
                                 start=True, stop=allfused)
                if not allfused:
                    nc.tensor.matmul(pb[:], ones_r[:], bb_sb[:],
                                     start=False, stop=True)
                xt = sb.tile([128, cout], F32, tag="xout")
                nc.vector.tensor_tensor(out=xt[:], in0=m_ap, in1=pb[:],
                                        op=ALU.add)
                nc.scalar.activation(xt[:], xt[:], AF.Relu)

                if l < 3:
                    # -sq_own for next layer's distance bias
                    scr = sb.tile([128, cout], F32, tag="sqscr")
                    sqc = sb.tile([128, 1], F32, tag="sqcol")
                    nc.scalar.activation(scr[:], xt[:], AF.Square,
                                         accum_out=sqc[:])
                    nc.scalar.activation(nsq_own_next[:, t:t + 1], sqc[:],
                                         AF.Copy, scale=-1.0)
                    nc.sync.dma_start(own_bounce[tsl, :], xt[:])

                # transpose own tile into the persistent xT chunks
                for cb, dst in enumerate(dsts):
                    rows = min(128, cout - 128 * cb)
                    pt = psM.tile([128, 128], F32, tag="psm")
                    nc.tensor.transpose(pt[:rows, :],
                                        xt[:, cb * 128:cb * 128 + rows],
                                        ident[:])
                    nc.scalar.copy(out=dst[:rows, tsl], in_=pt[:rows, :])

            # ---- between layers: AllGather + rebuild full-side state
            if l < 3:
                full_bounce = dram.tile([N, cout], F32, tag=f"fullb{l}")
                nc.gpsimd.collective_compute(
                    "AllGather", ALU.bypass, replica_groups=GROUPS,
                    ins=[own_bounce.opt()], outs=[full_bounce.opt()])

                cn = C_IN[l + 1]
                # cin=64 next layers get the -sq/2 row fused at row 64
                xT_full_next = ring.tile([cn + 1 if cn == 64 else cn, N],
                                         F32, tag="xTf")
                if cn == 64:
                    dest_row = xT_full_next[64:65, :]
                    nsqh_next = None
                else:
                    nsqh_next = persist.tile([1, N], F32, tag="nsqhr")
                    dest_row = nsqh_next[0:1, :]
                for j in range(NT_FULL):
                    jsl = slice(j * 128, (j + 1) * 128)
                    xf = sb.tile([128, cout], F32, tag="xfull")
                    nc.sync.dma_start(xf[:], full_bounce[jsl, :])
                    pt = psM.tile([128, 128], F32, tag="psm")
                    nc.tensor.transpose(pt[:cout, :], xf[:], ident[:])
                    nc.scalar.copy(out=xT_full_next[0:cout, jsl],
                                   in_=pt[:cout, :])
                    scr = sb.tile([128, cout], F32, tag="sqscr")
                    sqc = sb.tile([128, 1], F32, tag="sqcol")
                    nc.scalar.activation(scr[:], xf[:], AF.Square,
                                         accum_out=sqc[:])
                    sqh = sb.tile([128, 1], F32, tag="sqh")
                    nc.scalar.activation(sqh[:], sqc[:], AF.Copy, scale=-0.5)
                    nc.sync.dma_start(dest_row[0:1, jsl], sqh[:])

                xT_own = dsts[0][:]
                xT_full = xT_full_next
                nsqh_row = nsqh_next
                nsq_own = nsq_own_next

        x1T, x2T, x3T, x4Ta, x4Tb = xT_cls

        # ============ classifier pass A: g = relu(x_cat@Wg^T+b), gmax ============
        g_acc = persist.tile([128, EMB], F32, tag="gacc")
        for hh in range(2):
            nsl = slice(hh * 512, (hh + 1) * 512)
            gw = []
            for i, nm in enumerate(["gk0", "gk1", "gk2", "gk3", "gk4"]):
                gwt = sb.tile([list(io[nm].shape)[0], 512], F32, tag=f"gkh{i}",
                              bufs=1)
                eng = nc.gpsimd if io[nm].dtype != F32 else nc.sync
                eng.dma_start(gwt[:], io[nm][:, nsl])
                gw.append(gwt)
            gbt = sb.tile([1, 512], F32, tag="gbh", bufs=1)
            nc.sync.dma_start(gbt[:], io["gbias"][:, nsl])
            lhss = [x1T[0:64, :], x2T[0:64, :], x3T[:], x4Ta[:], x4Tb[:]]
            chunks = list(zip(lhss, gw))
            for t in range(NT_OWN):
                tsl = slice(t * 128, (t + 1) * 128)
                pg = psG.tile([128, 512], F32, tag="psg")
                for ci, (lhs_t, w_t) in enumerate(chunks):
                    nc.tensor.matmul(pg[:], lhs_t[:, tsl], w_t[:],
                                     start=(ci == 0), stop=False)
                nc.tensor.matmul(pg[:], ones_r[:], gbt[:],
                                 start=False, stop=True)
                gt = sb.tile([128, 512], F32, tag="gtile")
                nc.scalar.activation(gt[:], pg[:], AF.Relu)
                if t == 0:
                    nc.vector.tensor_copy(g_acc[:, nsl], gt[:])
                else:
                    nc.vector.tensor_tensor(out=g_acc[:, nsl],
                                            in0=g_acc[:, nsl],
                                            in1=gt[:], op=ALU.max)

        # gmax over points: transpose blocks + pool, then AllReduce(max)
        gmax8 = persist.tile([128, 8], F32, tag="gmax8")
        for b in range(8):
            pt = psM.tile([128, 128], F32, tag="psm")
            nc.tensor.transpose(pt[:], g_acc[:, b * 128:(b + 1) * 128], ident[:])
            nc.vector.pool_max(out=gmax8[:, b:b + 1], in_=pt[:])
        gmax_b = dram.tile([128, 8], F32, tag="gmaxb")
        gmax_rb = dram.tile([128, 8], F32, tag="gmaxrb")
        nc.sync.dma_start(gmax_b[:], gmax8[:])
        nc.gpsimd.collective_compute("AllReduce", ALU.max, replica_groups=GROUPS,
                                     ins=[gmax_b.opt()], outs=[gmax_rb.opt()])
        gmaxr = persist.tile([128, 8], F32, tag="gmaxr")
        nc.sync.dma_start(gmaxr[:], gmax_rb[:])

        # gv = Wgc' @ gmax + c1 bias -> [1, 256]
        c1b_sb = persist.tile([1, 256], F32, tag="c1b")
        nc.sync.dma_start(c1b_sb[:], io["c1b"][:])
        pgv = psM.tile([1, 256], F32, tag="psm")
        for b in range(8):
            wc = sb.tile([128, 256], F32, tag="wgch")
            nc.gpsimd.dma_start(wc[:], io["wgc"][b * 128:(b + 1) * 128, :])
            nc.tensor.matmul(pgv[:], gmaxr[:, b:b + 1], wc[:],
                             start=(b == 0), stop=False)
        nc.tensor.matmul(pgv[:], ones_r[0:1, 0:1], c1b_sb[:],
                         start=False, stop=True)
        gv = persist.tile([1, 256], F32, tag="gv")
        nc.scalar.copy(out=gv[:], in_=pgv[:])

        # ============ classifier pass B: c1 -> c2 -> logits ============
        wsb = {}
        for name in ["cx0", "cx1", "cx2", "cx3", "cx4", "c2wa", "c2wb", "c2b",
                     "clwa", "clwb", "clb"]:
            wt = persist.tile(list(io[name].shape), F32, tag=f"w_{name}")
            wload(wt[:], io[name])
            wsb[name] = wt

        for t in range(NT_OWN):
            tsl = slice(t * 128, (t + 1) * 128)
            p1 = psM.tile([128, 256], F32, tag="psm")
            chunks = [(x1T[0:64, :], "cx0"), (x2T[0:64, :], "cx1"),
                      (x3T[:], "cx2"), (x4Ta[:], "cx3"), (x4Tb[:], "cx4")]
            for ci, (lhs_t, wname) in enumerate(chunks):
                nc.tensor.matmul(p1[:], lhs_t[:, tsl], wsb[wname][:],
                                 start=(ci == 0), stop=False)
            nc.tensor.matmul(p1[:], ones_r[:], gv[:],
                             start=False, stop=True)
            c1t = sb.tile([128, 256], F32, tag="c1t")
            nc.scalar.activation(c1t[:], p1[:], AF.Relu)

            c1Ta = sb.tile([128, 128], F32, tag="c1Ta")
            c1Tb = sb.tile([128, 128], F32, tag="c1Tb")
            for cb, dstt in enumerate([c1Ta, c1Tb]):
                pt = psM.tile([128, 128], F32, tag="psm")
                nc.tensor.transpose(pt[:], c1t[:, cb * 128:(cb + 1) * 128],
                                    ident[:])
                nc.scalar.copy(out=dstt[:], in_=pt[:])

            p2 = psM.tile([128, 256], F32, tag="psm")
            nc.tensor.matmul(p2[:], c1Ta[:], wsb["c2wa"][:],
                             start=True, stop=False)
            nc.tensor.matmul(p2[:], c1Tb[:], wsb["c2wb"][:],
                             start=False, stop=False)
            nc.tensor.matmul(p2[:], ones_r[:], wsb["c2b"][:],
                             start=False, stop=True)
            c2t = sb.tile([128, 256], F32, tag="c2t")
            nc.scalar.activation(c2t[:], p2[:], AF.Relu)

            c2Ta = sb.tile([128, 128], F32, tag="c2Ta")
            c2Tb = sb.tile([128, 128], F32, tag="c2Tb")
            for cb, dstt in enumerate([c2Ta, c2Tb]):
                pt = psM.tile([128, 128], F32, tag="psm")
                nc.tensor.transpose(pt[:], c2t[:, cb * 128:(cb + 1) * 128],
                                    ident[:])
                nc.scalar.copy(out=dstt[:], in_=pt[:])

            p3 = psM.tile([128, NCLS], F32, tag="psm")
            nc.tensor.matmul(p3[:], c2Ta[:], wsb["clwa"][:],
                             start=True, stop=False)
            nc.tensor.matmul(p3[:], c2Tb[:], wsb["clwb"][:],
                             start=False, stop=False)
            nc.tensor.matmul(p3[:], ones_r[:], wsb["clb"][:],
                             start=False, stop=True)
            lo = sb.tile([128, NCLS], F32, tag="lot")
            nc.scalar.copy(out=lo[:], in_=p3[:])
            nc.sync.dma_start(io["logits"][tsl, :], lo[:])


# ======================= host side =======================

def _np(x):
    return np.asarray(x, dtype=np.float32)


def _fold_bn(bnp):
    s = _np(bnp["gamma"]) / np.sqrt(_np(bnp["var"]) + EPS)
    b = _np(bnp["beta"]) - _np(bnp["mean"]) * s
    return s, b


def _weight_inputs(params):
    io = {}
    for l in range(4):
        w = _np(params[f"ec{l + 1}_w"])          # [C_out, 2C]
        s, bias = _fold_bn(params[f"ec{l + 1}_bn"])
        C = C_IN[l]
        Wn, Wc = w[:, :C], w[:, C:]
        An = s[:, None] * Wn
        Bw = s[:, None] * (Wc - Wn)
        io[f"aw{l}"] = np.ascontiguousarray(An.T).astype(
            np.float16 if l == 3 else np.float32)
        if l == 0:
            io["b1rhs"] = np.concatenate([Bw.T, bias[None, :]], 0)
        else:
            io[f"bw{l}"] = np.ascontiguousarray(Bw.T).astype(
                np.float16 if l == 3 else np.float32)
            io[f"bb{l}"] = bias[None, :].copy()
    sg, bg = _fold_bn(params["g_bn"])
    WgT = np.ascontiguousarray((sg[:, None] * _np(params["g_w"])).T)  # [512, 1024]
    bounds = [0, 64, 128, 256, 384, 512]
    for i in range(5):
        io[f"gk{i}"] = np.ascontiguousarray(
            WgT[bounds[i]:bounds[i + 1], :]).astype(np.float16)
    io["gbias"] = bg[None, :].copy()
    s1, b1 = _fold_bn(params["c1_bn"])
    W1T = np.ascontiguousarray((s1[:, None] * _np(params["c1_w"])).T)  # [1536, 256]
    for i in range(5):
        io[f"cx{i}"] = np.ascontiguousarray(
            W1T[bounds[i]:bounds[i + 1], :]).astype(np.float16)
    io["wgc"] = np.ascontiguousarray(W1T[512:, :]).astype(np.float16)
    io["c1b"] = b1[None, :].copy()
    s2, b2 = _fold_bn(params["c2_bn"])
    W2T = np.ascontiguousarray((s2[:, None] * _np(params["c2_w"])).T)  # [256, 256]
    io["c2wa"], io["c2wb"] = (W2T[:128].astype(np.float16),
                              W2T[128:].astype(np.float16))
    io["c2b"] = b2[None, :].copy()
    WcT = np.ascontiguousarray(_np(params["cls_w"]).T)                 # [256, 50]
    io["clwa"], io["clwb"] = (WcT[:128].astype(np.float16),
                              WcT[128:].astype(np.float16))
    io["clb"] = _np(params["cls_b"])[None, :].copy()
    return io


def _get_program():
    if "nc" not in _CACHE:
        _CACHE["nc"] = build_program()
    return _CACHE["nc"]


def _get_runner():
    """Build the sharded PJRT callable ONCE; reuse across kernel() calls
    (run_bass_via_pjrt re-traces every call, which costs ~2s wall)."""
    if "runner" in _CACHE:
        return _CACHE["runner"]
    import jax
    from jax.experimental.shard_map import shard_map
    from jax.sharding import Mesh, PartitionSpec
    from concourse import bass2jax

    nc = _get_program()
    bass2jax.install_neuronx_cc_hook()

    partition_name = (nc.partition_id_tensor.name
                      if nc.partition_id_tensor else None)
    in_names, out_names, out_avals, zero_shapes = [], [], [], []
    for alloc in nc.m.functions[0].allocations:
        if not isinstance(alloc, mybir.MemoryLocationSet):
            continue
        name = alloc.memorylocations[0].name
        if alloc.kind == "ExternalInput":
            if name != partition_name:
                in_names.append(name)
        elif alloc.kind == "ExternalOutput":
            shape = tuple(alloc.tensor_shape)
            dtype = mybir.dt.np(alloc.dtype)
            out_names.append(name)
            out_avals.append(jax.core.ShapedArray(shape, dtype))
            zero_shapes.append((shape, dtype))
    n_params = len(in_names)
    n_outs = len(out_names)
    all_in = in_names + out_names + ([partition_name] if partition_name else [])
    donate = tuple(range(n_params, n_params + n_outs))

    def _bass_body(*args):
        operands = list(args)
        if partition_name is not None:
            operands.append(bass2jax.partition_id_tensor())
        outs = bass2jax._bass_exec_p.bind(
            *operands,
            out_avals=tuple(out_avals),
            in_names=tuple(all_in),
            out_names=tuple(out_names),
            lowering_input_output_aliases=(),
            sim_require_finite=True,
            sim_require_nnan=True,
            nc=nc,
        )
        return tuple(outs)

    devices = jax.devices()[:NCORES]
    mesh = Mesh(np.asarray(devices), ("core",))
    _CACHE["mesh"] = mesh
    in_specs = (PartitionSpec("core"),) * (n_params + n_outs)
    out_specs = (PartitionSpec("core"),) * n_outs
    sharded = jax.jit(
        shard_map(_bass_body, mesh=mesh, in_specs=in_specs,
                  out_specs=out_specs, check_rep=False),
        donate_argnums=donate, keep_unused=True)
    _CACHE["runner"] = (sharded, in_names, out_names, out_avals, zero_shapes)
    return _CACHE["runner"]


def _run(in_maps):
    import jax
    import hashlib
    import jax
    from jax.sharding import NamedSharding, PartitionSpec
    sharded, in_names, out_names, out_avals, zero_shapes = _get_runner()
    PER_CORE = {"l1_lhsT", "l1_rhs", "nsq1"}
    sh = NamedSharding(_CACHE["mesh"], PartitionSpec("core"))

    # device-cache the (identical-across-calls) weight uploads
    wnames = [n for n in in_names if n not in PER_CORE]
    hsh = hashlib.md5()
    for n in wnames:
        hsh.update(np.asarray(in_maps[0][n]).tobytes())
    key = hsh.hexdigest()
    if _CACHE.get("dev_w_key") != key:
        _CACHE["dev_w"] = {
            n: jax.device_put(
                np.concatenate([np.asarray(in_maps[c][n])
                                for c in range(NCORES)], axis=0), sh)
            for n in wnames}
        _CACHE["dev_w_key"] = key

    concat_in = [
        np.concatenate([np.asarray(in_maps[c][name]) for c in range(NCORES)],
                       axis=0)
        if name in PER_CORE else _CACHE["dev_w"][name]
        for name in in_names]
    concat_zeros = [np.zeros((NCORES * s[0], *s[1:]), d)
                    for (s, d) in zero_shapes]
    try:
        out_arrs = jax.block_until_ready(sharded(*concat_in, *concat_zeros))
    except Exception:
        concat_zeros = [np.zeros((NCORES * s[0], *s[1:]), d)
                        for (s, d) in zero_shapes]
        out_arrs = jax.block_until_ready(sharded(*concat_in, *concat_zeros))
    results = []
    for c in range(NCORES):
        results.append({
            name: np.asarray(out_arrs[i]).reshape(
                NCORES, *out_avals[i].shape)[c]
            for i, name in enumerate(out_names)})
    return results


def kernel(xyz, params):
    xyz = _np(xyz)                                # [4, 4096, 3]
    wio = _weight_inputs(params)

    in_maps = []
    for core in range(NCORES):
        b, h = core // 2, core % 2
        cloud = xyz[b]                            # [4096, 3]
        own = cloud[h * HALF:(h + 1) * HALF]
        sq_full = (cloud * cloud).sum(-1)         # [4096]
        sq_own = sq_full[h * HALF:(h + 1) * HALF]
        im = dict(wio)
        im["l1_lhsT"] = np.concatenate(
            [own.T, np.ones((1, HALF), np.float32)], 0)
        im["l1_rhs"] = np.concatenate(
            [cloud.T, -0.5 * sq_full[None, :]], 0).astype(np.float32)
        im["nsq1"] = np.ascontiguousarray(
            (-sq_own).reshape(NT_OWN, 128).T)
        in_maps.append(im)

    import time
    t0 = time.perf_counter()
    results = _run(in_maps)
    _CACHE["last_exec_s"] = time.perf_counter() - t0
    out = np.zeros((B, N, NCLS), np.float32)
    for core in range(NCORES):
        b, h = core // 2, core % 2
        out[b, h * HALF:(h + 1) * HALF] = results[core]["logits"]
    _CACHE["last_results"] = results
    return out


# revision 17
# speedup vs baseline: 2.5498x; 1.1853x over previous
"""DGCNN segmentation forward pass on 8 Trainium2 NeuronCores.

Sharding: core c handles half h=c%2 of cloud b=c//2 (2048 points each).
Within a pair {2b, 2b+1}: AllGather of per-layer features (KNN and the
neighbor gathers need the full cloud), AllReduce-max for the global
embedding. Everything else is local. Rank order == global row order, so
the AllGather output is the canonical full cloud on both cores.

EdgeConv factorization (BN folded into conv weights on the host):
  max_k relu(bn(W·[x_j - x_i; x_i]))
  = relu( maxpool_{j in knn(i)} (x_j @ An^T) + x_i @ Bw^T + bias )
with An = s*Wn, Bw = s*(Wc - Wn). Each layer is then: one dense matmul
over all points (A'), a KNN top-20, a 20-row gather+maxpool of A', and a
per-point matmul (B'').

KNN: PE computes psum = x_i·x_j - sq_j/2 (ones-row fused into the
matmul), ACT emits negd = 2*psum - sq_i = -(d^2). Top-24 of negd per row
via DVE max8/max_index/match_replace; slot 0 is always self
(negd[i,i] ~ 0 is the row max), slots 1..20 are the 20 nearest
neighbors. top_k tie-break (lowest index) matches the reference.
"""
import sys
sys.path.insert(0, '/opt/trn_rl_repo')

import numpy as np

import concourse.bass as bass
import concourse.mybir as mybir
import concourse.tile as tile
from concourse import bacc
from concourse.bass_utils import run_bass_kernel_spmd

B, N, K, NCLS, EMB = 4, 4096, 20, 50, 1024
HALF = N // 2
NT_OWN = HALF // 128     # 16
NT_FULL = N // 128       # 32
NCORES = 8
EPS = 1e-5
F32 = mybir.dt.float32
U32 = mybir.dt.uint32
AF = mybir.ActivationFunctionType
ALU = mybir.AluOpType
GROUPS = [[0, 1], [2, 3], [4, 5], [6, 7]]
NEG_BIG = -1e30

C_IN = [3, 64, 64, 128]
C_OUT = [64, 64, 128, 256]

_CACHE = {}
DEBUG_IDX = False


def _max_over_k_inplace(nc, g, c):
    """In-place pairwise max over the K=20 dim of g [128, 20, c].
    Result lands in g[:, 0, :]. Writes always trail reads elementwise."""
    mm = lambda o, a, b: nc.vector.tensor_tensor(out=o, in0=a, in1=b, op=ALU.max)
    mm(g[:, 0:10, :], g[:, 0:20:2, :], g[:, 1:20:2, :])   # 20 -> 10
    mm(g[:, 0:5, :], g[:, 0:10:2, :], g[:, 1:10:2, :])    # 10 -> 5
    mm(g[:, 0:2, :], g[:, 0:4:2, :], g[:, 1:4:2, :])      # 4  -> 2
    mm(g[:, 0:1, :], g[:, 0:1, :], g[:, 1:2, :])          # 2  -> 1
    mm(g[:, 0:1, :], g[:, 0:1, :], g[:, 4:5, :])          # + leftover 5th
    return g[:, 0, :]


def build_program():
    nc = bacc.Bacc("TRN2", target_bir_lowering=False, debug=False,
                   num_devices=NCORES)

    def din(name, shape, dt=F32):
        return nc.dram_tensor(name, shape, dt, kind="ExternalInput")

    io = {}
    io["l1_lhsT"] = din("l1_lhsT", [4, HALF])    # [xyzT_own; ones]
    io["l1_rhs"] = din("l1_rhs", [4, N])         # [xyzT_full; -sq/2]
    io["nsq1"] = din("nsq1", [128, NT_OWN])      # -sq_own, col t = tile t
    F16 = mybir.dt.float16
    # layer-4 conv + classifier weights never feed a later KNN, so fp16
    # shipping (cast to fp32 on load) costs ~2e-4 relative, saves ~16MB
    # of tunnel transfer per call.
    for l in range(4):
        io[f"aw{l}"] = din(f"aw{l}", [C_IN[l], C_OUT[l]],
                           F16 if l == 3 else F32)
    io["b1rhs"] = din("b1rhs", [4, C_OUT[0]])    # [BwT1; bias1]
    for l in range(1, 4):
        io[f"bw{l}"] = din(f"bw{l}", [C_IN[l], C_OUT[l]],
                           F16 if l == 3 else F32)
        io[f"bb{l}"] = din(f"bb{l}", [1, C_OUT[l]])
    gkdims = [64, 64, 128, 128, 128]
    for i, d in enumerate(gkdims):
        io[f"gk{i}"] = din(f"gk{i}", [d, EMB], F16)
    io["gbias"] = din("gbias", [1, EMB])
    io["wgc"] = din("wgc", [EMB, 256], F16)
    for i, d in enumerate(gkdims):
        io[f"cx{i}"] = din(f"cx{i}", [d, 256], F16)
    io["c1b"] = din("c1b", [1, 256])
    io["c2wa"] = din("c2wa", [128, 256], F16)
    io["c2wb"] = din("c2wb", [128, 256], F16)
    io["c2b"] = din("c2b", [1, 256])
    io["clwa"] = din("clwa", [128, NCLS], F16)
    io["clwb"] = din("clwb", [128, NCLS], F16)
    io["clb"] = din("clb", [1, NCLS])

    io["logits"] = nc.dram_tensor("logits", [HALF, NCLS], F32,
                                  kind="ExternalOutput")
    if DEBUG_IDX:
        for l in range(4):
            io[f"idx{l + 1}"] = nc.dram_tensor(f"idx{l + 1}", [HALF, 24], U32,
                                               kind="ExternalOutput")

    with tile.TileContext(nc) as tc:
        _body(nc, tc, io)
    nc.compile()
    return nc


def _body(nc, tc, io):
    from contextlib import ExitStack
    with ExitStack() as ctx:
        sb = ctx.enter_context(tc.tile_pool(name="sb", bufs=2))
        persist = ctx.enter_context(tc.tile_pool(name="persist", bufs=1))
        ring = ctx.enter_context(tc.tile_pool(name="ring", bufs=1))
        negd_p = ctx.enter_context(tc.tile_pool(name="negd", bufs=2))
        gp = ctx.enter_context(tc.tile_pool(name="gather", bufs=1))
        psD = ctx.enter_context(tc.tile_pool(name="psD", bufs=2, space="PSUM"))
        psM = ctx.enter_context(tc.tile_pool(name="psM", bufs=3, space="PSUM"))
        psG = ctx.enter_context(tc.tile_pool(name="psG", bufs=1, space="PSUM"))
        dram = ctx.enter_context(tc.tile_pool(name="dram", bufs=1, space="DRAM"))

        def wload(dst_ap, src_handle):
            eng = nc.gpsimd if src_handle.dtype != F32 else nc.sync
            eng.dma_start(dst_ap, src_handle[:])

        from concourse.masks import make_identity
        ident = persist.tile([128, 128], F32, tag="ident")
        make_identity(nc, ident[:])
        # one [1,128] ones row reused as the K=1 lhsT for every tile
        ones_r = persist.tile([1, 128], F32, tag="ones")
        nc.vector.memset(ones_r[:], 1.0)

        # layer-1 tensors from host (l1_rhs shares the xTf ring slot,
        # l1_lhsT shares the x2T slot: dead before x2T is written)
        l1rhs_sb = ring.tile([4, N], F32, tag="xTf")
        nc.sync.dma_start(l1rhs_sb[:], io["l1_rhs"][:])
        l1lhs_sb = persist.tile([4, HALF], F32, tag="l1x2")
        nc.sync.dma_start(l1lhs_sb[:], io["l1_lhsT"][:])
        nsq1_sb = persist.tile([128, NT_OWN], F32, tag="nsq1")
        nc.sync.dma_start(nsq1_sb[:], io["nsq1"][:])

        xT_own = l1lhs_sb          # [4, HALF] with ones row fused (layer 1)
        xT_full = l1rhs_sb         # [4, N] with -sq/2 row fused (layer 1)
        nsqh_row = None
        nsq_own = nsq1_sb

        xT_cls = [None] * 5        # x1T, x2T, x3T, x4Ta, x4Tb

        for l in range(4):
            cin, cout = C_IN[l], C_OUT[l]
            fused = (l == 0)

            # destination(s) for this layer's own-feature transposes
            if l == 0:
                x1T = persist.tile([65, HALF], F32, tag="x1T")
                nc.vector.memset(x1T[64:65, :], 1.0)
                dsts = [x1T]
                xT_cls[0] = x1T
            elif l == 1:
                x2T = persist.tile([65, HALF], F32, tag="l1x2")
                nc.vector.memset(x2T[64:65, :], 1.0)
                dsts = [x2T]
                xT_cls[1] = x2T
            elif l == 2:
                x3T = persist.tile([128, HALF], F32, tag="x3T")
                dsts = [x3T]
                xT_cls[2] = x3T
            else:
                x4Ta = persist.tile([128, HALF], F32, tag="x4Ta")
                x4Tb = persist.tile([128, HALF], F32, tag="x4Tb")
                dsts = [x4Ta, x4Tb]
                xT_cls[3], xT_cls[4] = x4Ta, x4Tb

            # ---- A' = x_full @ An^T -> DRAM [N, cout]
            a_rhs = persist.tile([cin, cout], F32, tag="awsb")
            wload(a_rhs[:], io[f"aw{l}"])
            A_dram = dram.tile([N, cout], F32, tag=f"Adram{l}")
            for j in range(NT_FULL):
                jsl = slice(j * 128, (j + 1) * 128)
                pa = psM.tile([128, cout], F32, tag="psm")
                nc.tensor.matmul(pa[:], xT_full[0:cin, jsl], a_rhs[:],
                                 start=True, stop=True)
                asb = sb.tile([128, cout], F32, tag="asb")
                nc.scalar.copy(out=asb[:], in_=pa[:])
                nc.sync.dma_start(A_dram[jsl, :], asb[:])

            # ---- B'' weights
            if fused:
                b_rhs = persist.tile([4, cout], F32, tag="bwsb")
                nc.sync.dma_start(b_rhs[:], io["b1rhs"][:])
                bb_sb = None
            elif l < 3:
                # bias folded as row cin (pairs with the xT ones row)
                b_rhs = persist.tile([cin + 1, cout], F32, tag="bwsb")
                wload(b_rhs[0:cin, :], io[f"bw{l}"])
                nc.sync.dma_start(b_rhs[cin:cin + 1, :], io[f"bb{l}"][:])
                bb_sb = None
            else:
                b_rhs = persist.tile([cin, cout], F32, tag="bwsb")
                wload(b_rhs[:], io[f"bw{l}"])
                bb_sb = persist.tile([1, cout], F32, tag="bbsb")
                nc.sync.dma_start(bb_sb[:], io[f"bb{l}"][:])

            if l < 3:
                own_bounce = dram.tile([HALF, cout], F32, tag=f"ownb{l}")
                nsq_own_next = persist.tile([128, NT_OWN], F32, tag=f"nsqo{l % 2}")

            # l=0: host fused [xyzT;ones]x[xyzT;-sq/2]; l=1,2: device-fused
            # ones/-sq rows (kdim=65); l=3: separate ones x (-sq/2) matmul
            kdim = 4 if fused else (cin + 1 if l < 3 else cin)
            allfused = l < 3

            for t in range(NT_OWN):
                tsl = slice(t * 128, (t + 1) * 128)
                negd = negd_p.tile([128, N], F32, tag="negd")
                for ch in range(4):
                    csl = slice(ch * 1024, (ch + 1) * 1024)
                    pd = psD.tile([128, 1024], F32, tag="psd")
                    for sub in range(2):
                        ssl = slice(ch * 1024 + sub * 512,
                                    ch * 1024 + (sub + 1) * 512)
                        psl = slice(sub * 512, (sub + 1) * 512)
                        nc.tensor.matmul(pd[:, psl], xT_own[0:kdim, tsl],
                                         xT_full[0:kdim, ssl],
                                         start=True, stop=allfused)
                        if not allfused:
                            nc.tensor.matmul(pd[:, psl], ones_r[:],
                                             nsqh_row[:, ssl],
                                             start=False, stop=True)
                    nc.scalar.activation(negd[:, csl], pd[:], AF.Identity,
                                         bias=nsq_own[:, t:t + 1], scale=2.0)

                # top-24 (slot 0 = self)
                idx = sb.tile([128, 24], U32, tag="tkidx")
                vals = sb.tile([128, 24], F32, tag="tkvals")
                for r in range(3):
                    rsl = slice(r * 8, (r + 1) * 8)
                    nc.vector.max(out=vals[:, rsl], in_=negd[:])
                    nc.vector.max_index(out=idx[:, rsl], in_max=vals[:, rsl],
                                        in_values=negd[:])
                    if r < 2:
                        nc.vector.match_replace(out=negd[:],
                                                in_to_replace=vals[:, rsl],
                                                in_values=negd[:],
                                                imm_value=NEG_BIG)
                if DEBUG_IDX:
                    nc.sync.dma_start(io[f"idx{l + 1}"][tsl, :], idx[:])

                # gather 20 neighbor rows of A' and max-pool them
                g = gp.tile([128, K, cout], F32, tag="gbuf")
                for r in range(K):
                    nc.gpsimd.indirect_dma_start(
                        out=g[:, r, :], out_offset=None, in_=A_dram[:],
                        in_offset=bass.IndirectOffsetOnAxis(
                            ap=idx[:, 1 + r:2 + r], axis=0))
                m_ap = _max_over_k_inplace(nc, g, cout)

                # B'' + relu
                pb = psM.tile([128, cout], F32, tag="psm")
                nc.tensor.matmul(pb[:], xT_own[0:kdim, tsl], b_rhs[:],
                                 start=True, stop=allfused)
                if not allfused:
                    nc.tensor.matmul(pb[:], ones_r[:], bb_sb[:],
                                     start=False, stop=True)
                xt = sb.tile([128, cout], F32, tag="xout")
                nc.vector.tensor_tensor(out=xt[:], in0=m_ap, in1=pb[:],
                                        op=ALU.add)
                nc.scalar.activation(xt[:], xt[:], AF.Relu)

                if l < 3:
                    # -sq_own for next layer's distance bias
                    scr = sb.tile([128, cout], F32, tag="sqscr")
                    sqc = sb.tile([128, 1], F32, tag="sqcol")
                    nc.scalar.activation(scr[:], xt[:], AF.Square,
                                         accum_out=sqc[:])
                    nc.scalar.activation(nsq_own_next[:, t:t + 1], sqc[:],
                                         AF.Copy, scale=-1.0)
                    nc.sync.dma_start(own_bounce[tsl, :], xt[:])

                # transpose own tile into the persistent xT chunks
                for cb, dst in enumerate(dsts):
                    rows = min(128, cout - 128 * cb)
                    pt = psM.tile([128, 128], F32, tag="psm")
                    nc.tensor.transpose(pt[:rows, :],
                                        xt[:, cb * 128:cb * 128 + rows],
                                        ident[:])
                    nc.scalar.copy(out=dst[:rows, tsl], in_=pt[:rows, :])

            # ---- between layers: AllGather + rebuild full-side state
            if l < 3:
                full_bounce = dram.tile([N, cout], F32, tag=f"fullb{l}")
                nc.gpsimd.collective_compute(
                    "AllGather", ALU.bypass, replica_groups=GROUPS,
                    ins=[own_bounce.opt()], outs=[full_bounce.opt()])

                cn = C_IN[l + 1]
                # cin=64 next layers get the -sq/2 row fused at row 64
                xT_full_next = ring.tile([cn + 1 if cn == 64 else cn, N],
                                         F32, tag="xTf")
                if cn == 64:
                    dest_row = xT_full_next[64:65, :]
                    nsqh_next = None
                else:
                    nsqh_next = persist.tile([1, N], F32, tag="nsqhr")
                    dest_row = nsqh_next[0:1, :]
                for j in range(NT_FULL):
                    jsl = slice(j * 128, (j + 1) * 128)
                    xf = sb.tile([128, cout], F32, tag="xfull")
                    nc.sync.dma_start(xf[:], full_bounce[jsl, :])
                    pt = psM.tile([128, 128], F32, tag="psm")
                    nc.tensor.transpose(pt[:cout, :], xf[:], ident[:])
                    nc.scalar.copy(out=xT_full_next[0:cout, jsl],
                                   in_=pt[:cout, :])
                    scr = sb.tile([128, cout], F32, tag="sqscr")
                    sqc = sb.tile([128, 1], F32, tag="sqcol")
                    nc.scalar.activation(scr[:], xf[:], AF.Square,
                                         accum_out=sqc[:])
                    sqh = sb.tile([128, 1], F32, tag="sqh")
                    nc.scalar.activation(sqh[:], sqc[:], AF.Copy, scale=-0.5)
                    nc.sync.dma_start(dest_row[0:1, jsl], sqh[:])

                xT_own = dsts[0][:]
                xT_full = xT_full_next
                nsqh_row = nsqh_next
                nsq_own = nsq_own_next

        x1T, x2T, x3T, x4Ta, x4Tb = xT_cls

        # ============ classifier pass A: g = relu(x_cat@Wg^T+b), gmax ============
        g_acc = persist.tile([128, EMB], F32, tag="gacc")
        for hh in range(2):
            nsl = slice(hh * 512, (hh + 1) * 512)
            gw = []
            for i, nm in enumerate(["gk0", "gk1", "gk2", "gk3", "gk4"]):
                gwt = sb.tile([list(io[nm].shape)[0], 512], F32, tag=f"gkh{i}",
                              bufs=1)
                eng = nc.gpsimd if io[nm].dtype != F32 else nc.sync
                eng.dma_start(gwt[:], io[nm][:, nsl])
                gw.append(gwt)
            gbt = sb.tile([1, 512], F32, tag="gbh", bufs=1)
            nc.sync.dma_start(gbt[:], io["gbias"][:, nsl])
            lhss = [x1T[0:64, :], x2T[0:64, :], x3T[:], x4Ta[:], x4Tb[:]]
            chunks = list(zip(lhss, gw))
            for t in range(NT_OWN):
                tsl = slice(t * 128, (t + 1) * 128)
                pg = psG.tile([128, 512], F32, tag="psg")
                for ci, (lhs_t, w_t) in enumerate(chunks):
                    nc.tensor.matmul(pg[:], lhs_t[:, tsl], w_t[:],
                                     start=(ci == 0), stop=False)
                nc.tensor.matmul(pg[:], ones_r[:], gbt[:],
                                 start=False, stop=True)
                gt = sb.tile([128, 512], F32, tag="gtile")
                nc.scalar.activation(gt[:], pg[:], AF.Relu)
                if t == 0:
                    nc.vector.tensor_copy(g_acc[:, nsl], gt[:])
                else:
                    nc.vector.tensor_tensor(out=g_acc[:, nsl],
                                            in0=g_acc[:, nsl],
                                            in1=gt[:], op=ALU.max)

        # gmax over points: transpose blocks + pool, then AllReduce(max)
        gmax8 = persist.tile([128, 8], F32, tag="gmax8")
        for b in range(8):
            pt = psM.tile([128, 128], F32, tag="psm")
            nc.tensor.transpose(pt[:], g_acc[:, b * 128:(b + 1) * 128], ident[:])
            nc.vector.pool_max(out=gmax8[:, b:b + 1], in_=pt[:])
        gmax_b = dram.tile([128, 8], F32, tag="gmaxb")
        gmax_rb = dram.tile([128, 8], F32, tag="gmaxrb")
        nc.sync.dma_start(gmax_b[:], gmax8[:])
        nc.gpsimd.collective_compute("AllReduce", ALU.max, replica_groups=GROUPS,
                                     ins=[gmax_b.opt()], outs=[gmax_rb.opt()])
        gmaxr = persist.tile([128, 8], F32, tag="gmaxr")
        nc.sync.dma_start(gmaxr[:], gmax_rb[:])

        # gv = Wgc' @ gmax + c1 bias -> [1, 256]
        c1b_sb = persist.tile([1, 256], F32, tag="c1b")
        nc.sync.dma_start(c1b_sb[:], io["c1b"][:])
        pgv = psM.tile([1, 256], F32, tag="psm")
        for b in range(8):
            wc = sb.tile([128, 256], F32, tag="wgch")
            nc.gpsimd.dma_start(wc[:], io["wgc"][b * 128:(b + 1) * 128, :])
            nc.tensor.matmul(pgv[:], gmaxr[:, b:b + 1], wc[:],
                             start=(b == 0), stop=False)
        nc.tensor.matmul(pgv[:], ones_r[0:1, 0:1], c1b_sb[:],
                         start=False, stop=True)
        gv = persist.tile([1, 256], F32, tag="gv")
        nc.scalar.copy(out=gv[:], in_=pgv[:])

        # ============ classifier pass B: c1 -> c2 -> logits ============
        wsb = {}
        for name in ["cx0", "cx1", "cx2", "cx3", "cx4", "c2wa", "c2wb", "c2b",
                     "clwa", "clwb", "clb"]:
            wt = persist.tile(list(io[name].shape), F32, tag=f"w_{name}")
            wload(wt[:], io[name])
            wsb[name] = wt

        for t in range(NT_OWN):
            tsl = slice(t * 128, (t + 1) * 128)
            p1 = psM.tile([128, 256], F32, tag="psm")
            chunks = [(x1T[0:64, :], "cx0"), (x2T[0:64, :], "cx1"),
                      (x3T[:], "cx2"), (x4Ta[:], "cx3"), (x4Tb[:], "cx4")]
            for ci, (lhs_t, wname) in enumerate(chunks):
                nc.tensor.matmul(p1[:], lhs_t[:, tsl], wsb[wname][:],
                                 start=(ci == 0), stop=False)
            nc.tensor.matmul(p1[:], ones_r[:], gv[:],
                             start=False, stop=True)
            c1t = sb.tile([128, 256], F32, tag="c1t")
            nc.scalar.activation(c1t[:], p1[:], AF.Relu)

            c1Ta = sb.tile([128, 128], F32, tag="c1Ta")
            c1Tb = sb.tile([128, 128], F32, tag="c1Tb")
            for cb, dstt in enumerate([c1Ta, c1Tb]):
                pt = psM.tile([128, 128], F32, tag="psm")
                nc.tensor.transpose(pt[:], c1t[:, cb * 128:(cb + 1) * 128],
                                    ident[:])
                nc.scalar.copy(out=dstt[:], in_=pt[:])

            p2 = psM.tile([128, 256], F32, tag="psm")
            nc.tensor.matmul(p2[:], c1Ta[:], wsb["c2wa"][:],
                             start=True, stop=False)
            nc.tensor.matmul(p2[:], c1Tb[:], wsb["c2wb"][:],
                             start=False, stop=False)
            nc.tensor.matmul(p2[:], ones_r[:], wsb["c2b"][:],
                             start=False, stop=True)
            c2t = sb.tile([128, 256], F32, tag="c2t")
            nc.scalar.activation(c2t[:], p2[:], AF.Relu)

            c2Ta = sb.tile([128, 128], F32, tag="c2Ta")
            c2Tb = sb.tile([128, 128], F32, tag="c2Tb")
            for cb, dstt in enumerate([c2Ta, c2Tb]):
                pt = psM.tile([128, 128], F32, tag="psm")
                nc.tensor.transpose(pt[:], c2t[:, cb * 128:(cb + 1) * 128],
                                    ident[:])
                nc.scalar.copy(out=dstt[:], in_=pt[:])

            p3 = psM.tile([128, NCLS], F32, tag="psm")
            nc.tensor.matmul(p3[:], c2Ta[:], wsb["clwa"][:],
                             start=True, stop=False)
            nc.tensor.matmul(p3[:], c2Tb[:], wsb["clwb"][:],
                             start=False, stop=False)
            nc.tensor.matmul(p3[:], ones_r[:], wsb["clb"][:],
                             start=False, stop=True)
            lo = sb.tile([128, NCLS], F32, tag="lot")
            nc.scalar.copy(out=lo[:], in_=p3[:])
            nc.sync.dma_start(io["logits"][tsl, :], lo[:])


# ======================= host side =======================

def _np(x):
    return np.asarray(x, dtype=np.float32)


def _fold_bn(bnp):
    s = _np(bnp["gamma"]) / np.sqrt(_np(bnp["var"]) + EPS)
    b = _np(bnp["beta"]) - _np(bnp["mean"]) * s
    return s, b


def _weight_inputs(params):
    io = {}
    for l in range(4):
        w = _np(params[f"ec{l + 1}_w"])          # [C_out, 2C]
        s, bias = _fold_bn(params[f"ec{l + 1}_bn"])
        C = C_IN[l]
        Wn, Wc = w[:, :C], w[:, C:]
        An = s[:, None] * Wn
        Bw = s[:, None] * (Wc - Wn)
        io[f"aw{l}"] = np.ascontiguousarray(An.T).astype(
            np.float16 if l == 3 else np.float32)
        if l == 0:
            io["b1rhs"] = np.concatenate([Bw.T, bias[None, :]], 0)
        else:
            io[f"bw{l}"] = np.ascontiguousarray(Bw.T).astype(
                np.float16 if l == 3 else np.float32)
            io[f"bb{l}"] = bias[None, :].copy()
    sg, bg = _fold_bn(params["g_bn"])
    WgT = np.ascontiguousarray((sg[:, None] * _np(params["g_w"])).T)  # [512, 1024]
    bounds = [0, 64, 128, 256, 384, 512]
    for i in range(5):
        io[f"gk{i}"] = np.ascontiguousarray(
            WgT[bounds[i]:bounds[i + 1], :]).astype(np.float16)
    io["gbias"] = bg[None, :].copy()
    s1, b1 = _fold_bn(params["c1_bn"])
    W1T = np.ascontiguousarray((s1[:, None] * _np(params["c1_w"])).T)  # [1536, 256]
    for i in range(5):
        io[f"cx{i}"] = np.ascontiguousarray(
            W1T[bounds[i]:bounds[i + 1], :]).astype(np.float16)
    io["wgc"] = np.ascontiguousarray(W1T[512:, :]).astype(np.float16)
    io["c1b"] = b1[None, :].copy()
    s2, b2 = _fold_bn(params["c2_bn"])
    W2T = np.ascontiguousarray((s2[:, None] * _np(params["c2_w"])).T)  # [256, 256]
    io["c2wa"], io["c2wb"] = (W2T[:128].astype(np.float16),
                              W2T[128:].astype(np.float16))
    io["c2b"] = b2[None, :].copy()
    WcT = np.ascontiguousarray(_np(params["cls_w"]).T)                 # [256, 50]
    io["clwa"], io["clwb"] = (WcT[:128].astype(np.float16),
                              WcT[128:].astype(np.float16))
    io["clb"] = _np(params["cls_b"])[None, :].copy()
    return io


def _get_program():
    if "nc" not in _CACHE:
        _CACHE["nc"] = build_program()
    return _CACHE["nc"]


def _get_runner():
    """Build the sharded PJRT callable ONCE; reuse across kernel() calls
    (run_bass_via_pjrt re-traces every call, which costs ~2s wall)."""
    if "runner" in _CACHE:
        return _CACHE["runner"]
    import jax
    from jax.experimental.shard_map import shard_map
    from jax.sharding import Mesh, PartitionSpec
    from concourse import bass2jax

    nc = _get_program()
    bass2jax.install_neuronx_cc_hook()

    partition_name = (nc.partition_id_tensor.name
                      if nc.partition_id_tensor else None)
    in_names, out_names, out_avals, zero_shapes = [], [], [], []
    for alloc in nc.m.functions[0].allocations:
        if not isinstance(alloc, mybir.MemoryLocationSet):
            continue
        name = alloc.memorylocations[0].name
        if alloc.kind == "ExternalInput":
            if name != partition_name:
                in_names.append(name)
        elif alloc.kind == "ExternalOutput":
            shape = tuple(alloc.tensor_shape)
            dtype = mybir.dt.np(alloc.dtype)
            out_names.append(name)
            out_avals.append(jax.core.ShapedArray(shape, dtype))
            zero_shapes.append((shape, dtype))
    n_params = len(in_names)
    n_outs = len(out_names)
    all_in = in_names + out_names + ([partition_name] if partition_name else [])
    donate = tuple(range(n_params, n_params + n_outs))

    def _bass_body(*args):
        operands = list(args)
        if partition_name is not None:
            operands.append(bass2jax.partition_id_tensor())
        outs = bass2jax._bass_exec_p.bind(
            *operands,
            out_avals=tuple(out_avals),
            in_names=tuple(all_in),
            out_names=tuple(out_names),
            lowering_input_output_aliases=(),
            sim_require_finite=True,
            sim_require_nnan=True,
            nc=nc,
        )
        return tuple(outs)

    devices = jax.devices()[:NCORES]
    mesh = Mesh(np.asarray(devices), ("core",))
    _CACHE["mesh"] = mesh
    in_specs = (PartitionSpec("core"),) * (n_params + n_outs)
    out_specs = (PartitionSpec("core"),) * n_outs
    sharded = jax.jit(
        shard_map(_bass_body, mesh=mesh, in_specs=in_specs,
                  out_specs=out_specs, check_rep=False),
        donate_argnums=donate, keep_unused=True)
    _CACHE["runner"] = (sharded, in_names, out_names, out_avals, zero_shapes)
    return _CACHE["runner"]


def _run(in_maps):
    import jax
    import hashlib
    import jax
    from jax.sharding import NamedSharding, PartitionSpec
    sharded, in_names, out_names, out_avals, zero_shapes = _get_runner()
    PER_CORE = {"l1_lhsT", "l1_rhs", "nsq1"}
    sh = NamedSharding(_CACHE["mesh"], PartitionSpec("core"))

    # device-cache the (identical-across-calls) weight uploads
    wnames = [n for n in in_names if n not in PER_CORE]
    hsh = hashlib.md5()
    for n in wnames:
        hsh.update(np.asarray(in_maps[0][n]).tobytes())
    key = hsh.hexdigest()
    if _CACHE.get("dev_w_key") != key:
        _CACHE["dev_w"] = {
            n: jax.device_put(
                np.concatenate([np.asarray(in_maps[c][n])
                                for c in range(NCORES)], axis=0), sh)
            for n in wnames}
        _CACHE["dev_w_key"] = key

    concat_in = [
        np.concatenate([np.asarray(in_maps[c][name]) for c in range(NCORES)],
                       axis=0)
        if name in PER_CORE else _CACHE["dev_w"][name]
        for name in in_names]
    # donate last call's (fully-overwritten) outputs as this call's output
    # buffers; falls back to fresh zeros on the first call
    donbufs = _CACHE.pop("prev_outs", None)
    if donbufs is None:
        donbufs = [np.zeros((NCORES * s[0], *s[1:]), d)
                   for (s, d) in zero_shapes]
    try:
        out_arrs = jax.block_until_ready(sharded(*concat_in, *donbufs))
    except Exception:
        donbufs = [np.zeros((NCORES * s[0], *s[1:]), d)
                   for (s, d) in zero_shapes]
        out_arrs = jax.block_until_ready(sharded(*concat_in, *donbufs))
    results = []
    for c in range(NCORES):
        results.append({
            name: np.asarray(out_arrs[i]).reshape(
                NCORES, *out_avals[i].shape)[c]
            for i, name in enumerate(out_names)})
    _CACHE["prev_outs"] = list(out_arrs)
    return results


def kernel(xyz, params):
    xyz = _np(xyz)                                # [4, 4096, 3]
    wio = _weight_inputs(params)

    in_maps = []
    for core in range(NCORES):
        b, h = core // 2, core % 2
        cloud = xyz[b]                            # [4096, 3]
        own = cloud[h * HALF:(h + 1) * HALF]
        sq_full = (cloud * cloud).sum(-1)         # [4096]
        sq_own = sq_full[h * HALF:(h + 1) * HALF]
        im = dict(wio)
        im["l1_lhsT"] = np.concatenate(
            [own.T, np.ones((1, HALF), np.float32)], 0)
        im["l1_rhs"] = np.concatenate(
            [cloud.T, -0.5 * sq_full[None, :]], 0).astype(np.float32)
        im["nsq1"] = np.ascontiguousarray(
            (-sq_own).reshape(NT_OWN, 128).T)
        in_maps.append(im)

    import time
    t0 = time.perf_counter()
    results = _run(in_maps)
    _CACHE["last_exec_s"] = time.perf_counter() - t0
    out = np.zeros((B, N, NCLS), np.float32)
    for core in range(NCORES):
        b, h = core // 2, core % 2
        out[b, h * HALF:(h + 1) * HALF] = results[core]["logits"]
    _CACHE["last_results"] = results
    return out


# revision 18
# speedup vs baseline: 3.2323x; 1.2676x over previous
"""DGCNN segmentation forward pass on 8 Trainium2 NeuronCores.

Sharding: core c handles half h=c%2 of cloud b=c//2 (2048 points each).
Within a pair {2b, 2b+1}: AllGather of per-layer features (KNN and the
neighbor gathers need the full cloud), AllReduce-max for the global
embedding. Everything else is local. Rank order == global row order, so
the AllGather output is the canonical full cloud on both cores.

EdgeConv factorization (BN folded into conv weights on the host):
  max_k relu(bn(W·[x_j - x_i; x_i]))
  = relu( maxpool_{j in knn(i)} (x_j @ An^T) + x_i @ Bw^T + bias )
with An = s*Wn, Bw = s*(Wc - Wn). Each layer is then: one dense matmul
over all points (A'), a KNN top-20, a 20-row gather+maxpool of A', and a
per-point matmul (B'').

KNN: PE computes psum = x_i·x_j - sq_j/2 (ones-row fused into the
matmul), ACT emits negd = 2*psum - sq_i = -(d^2). Top-24 of negd per row
via DVE max8/max_index/match_replace; slot 0 is always self
(negd[i,i] ~ 0 is the row max), slots 1..20 are the 20 nearest
neighbors. top_k tie-break (lowest index) matches the reference.
"""
import sys
sys.path.insert(0, '/opt/trn_rl_repo')

import numpy as np

import concourse.bass as bass
import concourse.mybir as mybir
import concourse.tile as tile
from concourse import bacc
from concourse.bass_utils import run_bass_kernel_spmd

B, N, K, NCLS, EMB = 4, 4096, 20, 50, 1024
HALF = N // 2
NT_OWN = HALF // 128     # 16
NT_FULL = N // 128       # 32
NCORES = 8
EPS = 1e-5
F32 = mybir.dt.float32
U32 = mybir.dt.uint32
AF = mybir.ActivationFunctionType
ALU = mybir.AluOpType
GROUPS = [[0, 1], [2, 3], [4, 5], [6, 7]]
NEG_BIG = -1e30

C_IN = [3, 64, 64, 128]
C_OUT = [64, 64, 128, 256]

_CACHE = {}
DEBUG_IDX = False


def _max_over_k_inplace(nc, g, c):
    """In-place pairwise max over the K=20 dim of g [128, 20, c].
    Result lands in g[:, 0, :]. Writes always trail reads elementwise."""
    mm = lambda o, a, b: nc.vector.tensor_tensor(out=o, in0=a, in1=b, op=ALU.max)
    mm(g[:, 0:10, :], g[:, 0:20:2, :], g[:, 1:20:2, :])   # 20 -> 10
    mm(g[:, 0:5, :], g[:, 0:10:2, :], g[:, 1:10:2, :])    # 10 -> 5
    mm(g[:, 0:2, :], g[:, 0:4:2, :], g[:, 1:4:2, :])      # 4  -> 2
    mm(g[:, 0:1, :], g[:, 0:1, :], g[:, 1:2, :])          # 2  -> 1
    mm(g[:, 0:1, :], g[:, 0:1, :], g[:, 4:5, :])          # + leftover 5th
    return g[:, 0, :]


def build_program():
    nc = bacc.Bacc("TRN2", target_bir_lowering=False, debug=False,
                   num_devices=NCORES)

    def din(name, shape, dt=F32):
        return nc.dram_tensor(name, shape, dt, kind="ExternalInput")

    io = {}
    io["l1_lhsT"] = din("l1_lhsT", [4, HALF])    # [xyzT_own; ones]
    io["l1_rhs"] = din("l1_rhs", [4, N])         # [xyzT_full; -sq/2]
    io["nsq1"] = din("nsq1", [128, NT_OWN])      # -sq_own, col t = tile t
    F16 = mybir.dt.float16
    # layer-4 conv + classifier weights never feed a later KNN, so fp16
    # shipping (cast to fp32 on load) costs ~2e-4 relative, saves ~16MB
    # of tunnel transfer per call.
    for l in range(4):
        io[f"aw{l}"] = din(f"aw{l}", [C_IN[l], C_OUT[l]],
                           F16 if l == 3 else F32)
    io["b1rhs"] = din("b1rhs", [4, C_OUT[0]])    # [BwT1; bias1]
    for l in range(1, 4):
        io[f"bw{l}"] = din(f"bw{l}", [C_IN[l], C_OUT[l]],
                           F16 if l == 3 else F32)
        io[f"bb{l}"] = din(f"bb{l}", [1, C_OUT[l]])
    gkdims = [64, 64, 128, 128, 128]
    for i, d in enumerate(gkdims):
        io[f"gk{i}"] = din(f"gk{i}", [d, EMB], F16)
    io["gbias"] = din("gbias", [1, EMB])
    io["wgc"] = din("wgc", [EMB, 256], F16)
    for i, d in enumerate(gkdims):
        io[f"cx{i}"] = din(f"cx{i}", [d, 256], F16)
    io["c1b"] = din("c1b", [1, 256])
    io["c2wa"] = din("c2wa", [128, 256], F16)
    io["c2wb"] = din("c2wb", [128, 256], F16)
    io["c2b"] = din("c2b", [1, 256])
    io["clwa"] = din("clwa", [128, NCLS], F16)
    io["clwb"] = din("clwb", [128, NCLS], F16)
    io["clb"] = din("clb", [1, NCLS])

    io["logits"] = nc.dram_tensor("logits", [HALF, NCLS], mybir.dt.float16,
                                  kind="ExternalOutput")
    if DEBUG_IDX:
        for l in range(4):
            io[f"idx{l + 1}"] = nc.dram_tensor(f"idx{l + 1}", [HALF, 24], U32,
                                               kind="ExternalOutput")

    with tile.TileContext(nc) as tc:
        _body(nc, tc, io)
    nc.compile()
    return nc


def _body(nc, tc, io):
    from contextlib import ExitStack
    with ExitStack() as ctx:
        sb = ctx.enter_context(tc.tile_pool(name="sb", bufs=2))
        persist = ctx.enter_context(tc.tile_pool(name="persist", bufs=1))
        ring = ctx.enter_context(tc.tile_pool(name="ring", bufs=1))
        negd_p = ctx.enter_context(tc.tile_pool(name="negd", bufs=2))
        gp = ctx.enter_context(tc.tile_pool(name="gather", bufs=1))
        psD = ctx.enter_context(tc.tile_pool(name="psD", bufs=2, space="PSUM"))
        psM = ctx.enter_context(tc.tile_pool(name="psM", bufs=3, space="PSUM"))
        psG = ctx.enter_context(tc.tile_pool(name="psG", bufs=1, space="PSUM"))
        dram = ctx.enter_context(tc.tile_pool(name="dram", bufs=1, space="DRAM"))

        def wload(dst_ap, src_handle):
            eng = nc.gpsimd if src_handle.dtype != F32 else nc.sync
            eng.dma_start(dst_ap, src_handle[:])

        from concourse.masks import make_identity
        ident = persist.tile([128, 128], F32, tag="ident")
        make_identity(nc, ident[:])
        # one [1,128] ones row reused as the K=1 lhsT for every tile
        ones_r = persist.tile([1, 128], F32, tag="ones")
        nc.vector.memset(ones_r[:], 1.0)

        # layer-1 tensors from host (l1_rhs shares the xTf ring slot,
        # l1_lhsT shares the x2T slot: dead before x2T is written)
        l1rhs_sb = ring.tile([4, N], F32, tag="xTf")
        nc.sync.dma_start(l1rhs_sb[:], io["l1_rhs"][:])
        l1lhs_sb = persist.tile([4, HALF], F32, tag="l1x2")
        nc.sync.dma_start(l1lhs_sb[:], io["l1_lhsT"][:])
        nsq1_sb = persist.tile([128, NT_OWN], F32, tag="nsq1")
        nc.sync.dma_start(nsq1_sb[:], io["nsq1"][:])

        xT_own = l1lhs_sb          # [4, HALF] with ones row fused (layer 1)
        xT_full = l1rhs_sb         # [4, N] with -sq/2 row fused (layer 1)
        nsqh_row = None
        nsq_own = nsq1_sb

        xT_cls = [None] * 5        # x1T, x2T, x3T, x4Ta, x4Tb

        for l in range(4):
            cin, cout = C_IN[l], C_OUT[l]
            fused = (l == 0)

            # destination(s) for this layer's own-feature transposes
            if l == 0:
                x1T = persist.tile([65, HALF], F32, tag="x1T")
                nc.vector.memset(x1T[64:65, :], 1.0)
                dsts = [x1T]
                xT_cls[0] = x1T
            elif l == 1:
                x2T = persist.tile([65, HALF], F32, tag="l1x2")
                nc.vector.memset(x2T[64:65, :], 1.0)
                dsts = [x2T]
                xT_cls[1] = x2T
            elif l == 2:
                x3T = persist.tile([128, HALF], F32, tag="x3T")
                dsts = [x3T]
                xT_cls[2] = x3T
            else:
                x4Ta = persist.tile([128, HALF], F32, tag="x4Ta")
                x4Tb = persist.tile([128, HALF], F32, tag="x4Tb")
                dsts = [x4Ta, x4Tb]
                xT_cls[3], xT_cls[4] = x4Ta, x4Tb

            # ---- A' = x_full @ An^T -> DRAM [N, cout]
            a_rhs = persist.tile([cin, cout], F32, tag="awsb")
            wload(a_rhs[:], io[f"aw{l}"])
            A_dram = dram.tile([N, cout], F32, tag=f"Adram{l}")
            for j in range(NT_FULL):
                jsl = slice(j * 128, (j + 1) * 128)
                pa = psM.tile([128, cout], F32, tag="psm")
                nc.tensor.matmul(pa[:], xT_full[0:cin, jsl], a_rhs[:],
                                 start=True, stop=True)
                asb = sb.tile([128, cout], F32, tag="asb")
                nc.scalar.copy(out=asb[:], in_=pa[:])
                nc.sync.dma_start(A_dram[jsl, :], asb[:])

            # ---- B'' weights
            if fused:
                b_rhs = persist.tile([4, cout], F32, tag="bwsb")
                nc.sync.dma_start(b_rhs[:], io["b1rhs"][:])
                bb_sb = None
            elif l < 3:
                # bias folded as row cin (pairs with the xT ones row)
                b_rhs = persist.tile([cin + 1, cout], F32, tag="bwsb")
                wload(b_rhs[0:cin, :], io[f"bw{l}"])
                nc.sync.dma_start(b_rhs[cin:cin + 1, :], io[f"bb{l}"][:])
                bb_sb = None
            else:
                b_rhs = persist.tile([cin, cout], F32, tag="bwsb")
                wload(b_rhs[:], io[f"bw{l}"])
                bb_sb = persist.tile([1, cout], F32, tag="bbsb")
                nc.sync.dma_start(bb_sb[:], io[f"bb{l}"][:])

            if l < 3:
                own_bounce = dram.tile([HALF, cout], F32, tag=f"ownb{l}")
                nsq_own_next = persist.tile([128, NT_OWN], F32, tag=f"nsqo{l % 2}")

            # l=0: host fused [xyzT;ones]x[xyzT;-sq/2]; l=1,2: device-fused
            # ones/-sq rows (kdim=65); l=3: separate ones x (-sq/2) matmul
            kdim = 4 if fused else (cin + 1 if l < 3 else cin)
            allfused = l < 3

            for t in range(NT_OWN):
                tsl = slice(t * 128, (t + 1) * 128)
                negd = negd_p.tile([128, N], F32, tag="negd")
                for ch in range(4):
                    csl = slice(ch * 1024, (ch + 1) * 1024)
                    pd = psD.tile([128, 1024], F32, tag="psd")
                    for sub in range(2):
                        ssl = slice(ch * 1024 + sub * 512,
                                    ch * 1024 + (sub + 1) * 512)
                        psl = slice(sub * 512, (sub + 1) * 512)
                        nc.tensor.matmul(pd[:, psl], xT_own[0:kdim, tsl],
                                         xT_full[0:kdim, ssl],
                                         start=True, stop=allfused)
                        if not allfused:
                            nc.tensor.matmul(pd[:, psl], ones_r[:],
                                             nsqh_row[:, ssl],
                                             start=False, stop=True)
                    nc.scalar.activation(negd[:, csl], pd[:], AF.Identity,
                                         bias=nsq_own[:, t:t + 1], scale=2.0)

                # top-24 (slot 0 = self)
                idx = sb.tile([128, 24], U32, tag="tkidx")
                vals = sb.tile([128, 24], F32, tag="tkvals")
                for r in range(3):
                    rsl = slice(r * 8, (r + 1) * 8)
                    nc.vector.max(out=vals[:, rsl], in_=negd[:])
                    nc.vector.max_index(out=idx[:, rsl], in_max=vals[:, rsl],
                                        in_values=negd[:])
                    if r < 2:
                        nc.vector.match_replace(out=negd[:],
                                                in_to_replace=vals[:, rsl],
                                                in_values=negd[:],
                                                imm_value=NEG_BIG)
                if DEBUG_IDX:
                    nc.sync.dma_start(io[f"idx{l + 1}"][tsl, :], idx[:])

                # gather 20 neighbor rows of A' and max-pool them
                g = gp.tile([128, K, cout], F32, tag="gbuf")
                for r in range(K):
                    nc.gpsimd.indirect_dma_start(
                        out=g[:, r, :], out_offset=None, in_=A_dram[:],
                        in_offset=bass.IndirectOffsetOnAxis(
                            ap=idx[:, 1 + r:2 + r], axis=0))
                m_ap = _max_over_k_inplace(nc, g, cout)

                # B'' + relu
                pb = psM.tile([128, cout], F32, tag="psm")
                nc.tensor.matmul(pb[:], xT_own[0:kdim, tsl], b_rhs[:],
                                 start=True, stop=allfused)
                if not allfused:
                    nc.tensor.matmul(pb[:], ones_r[:], bb_sb[:],
                                     start=False, stop=True)
                xt = sb.tile([128, cout], F32, tag="xout")
                nc.vector.tensor_tensor(out=xt[:], in0=m_ap, in1=pb[:],
                                        op=ALU.add)
                nc.scalar.activation(xt[:], xt[:], AF.Relu)

                if l < 3:
                    # -sq_own for next layer's distance bias
                    scr = sb.tile([128, cout], F32, tag="sqscr")
                    sqc = sb.tile([128, 1], F32, tag="sqcol")
                    nc.scalar.activation(scr[:], xt[:], AF.Square,
                                         accum_out=sqc[:])
                    nc.scalar.activation(nsq_own_next[:, t:t + 1], sqc[:],
                                         AF.Copy, scale=-1.0)
                    nc.sync.dma_start(own_bounce[tsl, :], xt[:])

                # transpose own tile into the persistent xT chunks
                for cb, dst in enumerate(dsts):
                    rows = min(128, cout - 128 * cb)
                    pt = psM.tile([128, 128], F32, tag="psm")
                    nc.tensor.transpose(pt[:rows, :],
                                        xt[:, cb * 128:cb * 128 + rows],
                                        ident[:])
                    nc.scalar.copy(out=dst[:rows, tsl], in_=pt[:rows, :])

            # ---- between layers: AllGather + rebuild full-side state
            if l < 3:
                full_bounce = dram.tile([N, cout], F32, tag=f"fullb{l}")
                nc.gpsimd.collective_compute(
                    "AllGather", ALU.bypass, replica_groups=GROUPS,
                    ins=[own_bounce.opt()], outs=[full_bounce.opt()])

                cn = C_IN[l + 1]
                # cin=64 next layers get the -sq/2 row fused at row 64
                xT_full_next = ring.tile([cn + 1 if cn == 64 else cn, N],
                                         F32, tag="xTf")
                if cn == 64:
                    dest_row = xT_full_next[64:65, :]
                    nsqh_next = None
                else:
                    nsqh_next = persist.tile([1, N], F32, tag="nsqhr")
                    dest_row = nsqh_next[0:1, :]
                for j in range(NT_FULL):
                    jsl = slice(j * 128, (j + 1) * 128)
                    xf = sb.tile([128, cout], F32, tag="xfull")
                    nc.sync.dma_start(xf[:], full_bounce[jsl, :])
                    pt = psM.tile([128, 128], F32, tag="psm")
                    nc.tensor.transpose(pt[:cout, :], xf[:], ident[:])
                    nc.scalar.copy(out=xT_full_next[0:cout, jsl],
                                   in_=pt[:cout, :])
                    scr = sb.tile([128, cout], F32, tag="sqscr")
                    sqc = sb.tile([128, 1], F32, tag="sqcol")
                    nc.scalar.activation(scr[:], xf[:], AF.Square,
                                         accum_out=sqc[:])
                    sqh = sb.tile([128, 1], F32, tag="sqh")
                    nc.scalar.activation(sqh[:], sqc[:], AF.Copy, scale=-0.5)
                    nc.sync.dma_start(dest_row[0:1, jsl], sqh[:])

                xT_own = dsts[0][:]
                xT_full = xT_full_next
                nsqh_row = nsqh_next
                nsq_own = nsq_own_next

        x1T, x2T, x3T, x4Ta, x4Tb = xT_cls

        # ============ classifier pass A: g = relu(x_cat@Wg^T+b), gmax ============
        g_acc = persist.tile([128, EMB], F32, tag="gacc")
        for hh in range(2):
            nsl = slice(hh * 512, (hh + 1) * 512)
            gw = []
            for i, nm in enumerate(["gk0", "gk1", "gk2", "gk3", "gk4"]):
                gwt = sb.tile([list(io[nm].shape)[0], 512], F32, tag=f"gkh{i}",
                              bufs=1)
                eng = nc.gpsimd if io[nm].dtype != F32 else nc.sync
                eng.dma_start(gwt[:], io[nm][:, nsl])
                gw.append(gwt)
            gbt = sb.tile([1, 512], F32, tag="gbh", bufs=1)
            nc.sync.dma_start(gbt[:], io["gbias"][:, nsl])
            lhss = [x1T[0:64, :], x2T[0:64, :], x3T[:], x4Ta[:], x4Tb[:]]
            chunks = list(zip(lhss, gw))
            for t in range(NT_OWN):
                tsl = slice(t * 128, (t + 1) * 128)
                pg = psG.tile([128, 512], F32, tag="psg")
                for ci, (lhs_t, w_t) in enumerate(chunks):
                    nc.tensor.matmul(pg[:], lhs_t[:, tsl], w_t[:],
                                     start=(ci == 0), stop=False)
                nc.tensor.matmul(pg[:], ones_r[:], gbt[:],
                                 start=False, stop=True)
                gt = sb.tile([128, 512], F32, tag="gtile")
                nc.scalar.activation(gt[:], pg[:], AF.Relu)
                if t == 0:
                    nc.vector.tensor_copy(g_acc[:, nsl], gt[:])
                else:
                    nc.vector.tensor_tensor(out=g_acc[:, nsl],
                                            in0=g_acc[:, nsl],
                                            in1=gt[:], op=ALU.max)

        # gmax over points: transpose blocks + pool, then AllReduce(max)
        gmax8 = persist.tile([128, 8], F32, tag="gmax8")
        for b in range(8):
            pt = psM.tile([128, 128], F32, tag="psm")
            nc.tensor.transpose(pt[:], g_acc[:, b * 128:(b + 1) * 128], ident[:])
            nc.vector.pool_max(out=gmax8[:, b:b + 1], in_=pt[:])
        gmax_b = dram.tile([128, 8], F32, tag="gmaxb")
        gmax_rb = dram.tile([128, 8], F32, tag="gmaxrb")
        nc.sync.dma_start(gmax_b[:], gmax8[:])
        nc.gpsimd.collective_compute("AllReduce", ALU.max, replica_groups=GROUPS,
                                     ins=[gmax_b.opt()], outs=[gmax_rb.opt()])
        gmaxr = persist.tile([128, 8], F32, tag="gmaxr")
        nc.sync.dma_start(gmaxr[:], gmax_rb[:])

        # gv = Wgc' @ gmax + c1 bias -> [1, 256]
        c1b_sb = persist.tile([1, 256], F32, tag="c1b")
        nc.sync.dma_start(c1b_sb[:], io["c1b"][:])
        pgv = psM.tile([1, 256], F32, tag="psm")
        for b in range(8):
            wc = sb.tile([128, 256], F32, tag="wgch")
            nc.gpsimd.dma_start(wc[:], io["wgc"][b * 128:(b + 1) * 128, :])
            nc.tensor.matmul(pgv[:], gmaxr[:, b:b + 1], wc[:],
                             start=(b == 0), stop=False)
        nc.tensor.matmul(pgv[:], ones_r[0:1, 0:1], c1b_sb[:],
                         start=False, stop=True)
        gv = persist.tile([1, 256], F32, tag="gv")
        nc.scalar.copy(out=gv[:], in_=pgv[:])

        # ============ classifier pass B: c1 -> c2 -> logits ============
        wsb = {}
        for name in ["cx0", "cx1", "cx2", "cx3", "cx4", "c2wa", "c2wb", "c2b",
                     "clwa", "clwb", "clb"]:
            wt = persist.tile(list(io[name].shape), F32, tag=f"w_{name}")
            wload(wt[:], io[name])
            wsb[name] = wt

        for t in range(NT_OWN):
            tsl = slice(t * 128, (t + 1) * 128)
            p1 = psM.tile([128, 256], F32, tag="psm")
            chunks = [(x1T[0:64, :], "cx0"), (x2T[0:64, :], "cx1"),
                      (x3T[:], "cx2"), (x4Ta[:], "cx3"), (x4Tb[:], "cx4")]
            for ci, (lhs_t, wname) in enumerate(chunks):
                nc.tensor.matmul(p1[:], lhs_t[:, tsl], wsb[wname][:],
                                 start=(ci == 0), stop=False)
            nc.tensor.matmul(p1[:], ones_r[:], gv[:],
                             start=False, stop=True)
            c1t = sb.tile([128, 256], F32, tag="c1t")
            nc.scalar.activation(c1t[:], p1[:], AF.Relu)

            c1Ta = sb.tile([128, 128], F32, tag="c1Ta")
            c1Tb = sb.tile([128, 128], F32, tag="c1Tb")
            for cb, dstt in enumerate([c1Ta, c1Tb]):
                pt = psM.tile([128, 128], F32, tag="psm")
                nc.tensor.transpose(pt[:], c1t[:, cb * 128:(cb + 1) * 128],
                                    ident[:])
                nc.scalar.copy(out=dstt[:], in_=pt[:])

            p2 = psM.tile([128, 256], F32, tag="psm")
            nc.tensor.matmul(p2[:], c1Ta[:], wsb["c2wa"][:],
                             start=True, stop=False)
            nc.tensor.matmul(p2[:], c1Tb[:], wsb["c2wb"][:],
                             start=False, stop=False)
            nc.tensor.matmul(p2[:], ones_r[:], wsb["c2b"][:],
                             start=False, stop=True)
            c2t = sb.tile([128, 256], F32, tag="c2t")
            nc.scalar.activation(c2t[:], p2[:], AF.Relu)

            c2Ta = sb.tile([128, 128], F32, tag="c2Ta")
            c2Tb = sb.tile([128, 128], F32, tag="c2Tb")
            for cb, dstt in enumerate([c2Ta, c2Tb]):
                pt = psM.tile([128, 128], F32, tag="psm")
                nc.tensor.transpose(pt[:], c2t[:, cb * 128:(cb + 1) * 128],
                                    ident[:])
                nc.scalar.copy(out=dstt[:], in_=pt[:])

            p3 = psM.tile([128, NCLS], F32, tag="psm")
            nc.tensor.matmul(p3[:], c2Ta[:], wsb["clwa"][:],
                             start=True, stop=False)
            nc.tensor.matmul(p3[:], c2Tb[:], wsb["clwb"][:],
                             start=False, stop=False)
            nc.tensor.matmul(p3[:], ones_r[:], wsb["clb"][:],
                             start=False, stop=True)
            lo = sb.tile([128, NCLS], mybir.dt.float16, tag="lot")
            nc.scalar.copy(out=lo[:], in_=p3[:])
            nc.sync.dma_start(io["logits"][tsl, :], lo[:])


# ======================= host side =======================

def _np(x):
    return np.asarray(x, dtype=np.float32)


def _fold_bn(bnp):
    s = _np(bnp["gamma"]) / np.sqrt(_np(bnp["var"]) + EPS)
    b = _np(bnp["beta"]) - _np(bnp["mean"]) * s
    return s, b


def _weight_inputs(params):
    io = {}
    for l in range(4):
        w = _np(params[f"ec{l + 1}_w"])          # [C_out, 2C]
        s, bias = _fold_bn(params[f"ec{l + 1}_bn"])
        C = C_IN[l]
        Wn, Wc = w[:, :C], w[:, C:]
        An = s[:, None] * Wn
        Bw = s[:, None] * (Wc - Wn)
        io[f"aw{l}"] = np.ascontiguousarray(An.T).astype(
            np.float16 if l == 3 else np.float32)
        if l == 0:
            io["b1rhs"] = np.concatenate([Bw.T, bias[None, :]], 0)
        else:
            io[f"bw{l}"] = np.ascontiguousarray(Bw.T).astype(
                np.float16 if l == 3 else np.float32)
            io[f"bb{l}"] = bias[None, :].copy()
    sg, bg = _fold_bn(params["g_bn"])
    WgT = np.ascontiguousarray((sg[:, None] * _np(params["g_w"])).T)  # [512, 1024]
    bounds = [0, 64, 128, 256, 384, 512]
    for i in range(5):
        io[f"gk{i}"] = np.ascontiguousarray(
            WgT[bounds[i]:bounds[i + 1], :]).astype(np.float16)
    io["gbias"] = bg[None, :].copy()
    s1, b1 = _fold_bn(params["c1_bn"])
    W1T = np.ascontiguousarray((s1[:, None] * _np(params["c1_w"])).T)  # [1536, 256]
    for i in range(5):
        io[f"cx{i}"] = np.ascontiguousarray(
            W1T[bounds[i]:bounds[i + 1], :]).astype(np.float16)
    io["wgc"] = np.ascontiguousarray(W1T[512:, :]).astype(np.float16)
    io["c1b"] = b1[None, :].copy()
    s2, b2 = _fold_bn(params["c2_bn"])
    W2T = np.ascontiguousarray((s2[:, None] * _np(params["c2_w"])).T)  # [256, 256]
    io["c2wa"], io["c2wb"] = (W2T[:128].astype(np.float16),
                              W2T[128:].astype(np.float16))
    io["c2b"] = b2[None, :].copy()
    WcT = np.ascontiguousarray(_np(params["cls_w"]).T)                 # [256, 50]
    io["clwa"], io["clwb"] = (WcT[:128].astype(np.float16),
                              WcT[128:].astype(np.float16))
    io["clb"] = _np(params["cls_b"])[None, :].copy()
    return io


def _get_program():
    if "nc" not in _CACHE:
        _CACHE["nc"] = build_program()
    return _CACHE["nc"]


def _get_runner():
    """Build the sharded PJRT callable ONCE; reuse across kernel() calls
    (run_bass_via_pjrt re-traces every call, which costs ~2s wall)."""
    if "runner" in _CACHE:
        return _CACHE["runner"]
    import jax
    from jax.experimental.shard_map import shard_map
    from jax.sharding import Mesh, PartitionSpec
    from concourse import bass2jax

    nc = _get_program()
    bass2jax.install_neuronx_cc_hook()

    partition_name = (nc.partition_id_tensor.name
                      if nc.partition_id_tensor else None)
    in_names, out_names, out_avals, zero_shapes = [], [], [], []
    for alloc in nc.m.functions[0].allocations:
        if not isinstance(alloc, mybir.MemoryLocationSet):
            continue
        name = alloc.memorylocations[0].name
        if alloc.kind == "ExternalInput":
            if name != partition_name:
                in_names.append(name)
        elif alloc.kind == "ExternalOutput":
            shape = tuple(alloc.tensor_shape)
            dtype = mybir.dt.np(alloc.dtype)
            out_names.append(name)
            out_avals.append(jax.core.ShapedArray(shape, dtype))
            zero_shapes.append((shape, dtype))
    n_params = len(in_names)
    n_outs = len(out_names)
    all_in = in_names + out_names + ([partition_name] if partition_name else [])
    donate = tuple(range(n_params, n_params + n_outs))

    def _bass_body(*args):
        operands = list(args)
        if partition_name is not None:
            operands.append(bass2jax.partition_id_tensor())
        outs = bass2jax._bass_exec_p.bind(
            *operands,
            out_avals=tuple(out_avals),
            in_names=tuple(all_in),
            out_names=tuple(out_names),
            lowering_input_output_aliases=(),
            sim_require_finite=True,
            sim_require_nnan=True,
            nc=nc,
        )
        return tuple(outs)

    devices = jax.devices()[:NCORES]
    mesh = Mesh(np.asarray(devices), ("core",))
    _CACHE["mesh"] = mesh
    in_specs = (PartitionSpec("core"),) * (n_params + n_outs)
    out_specs = (PartitionSpec("core"),) * n_outs
    sharded = jax.jit(
        shard_map(_bass_body, mesh=mesh, in_specs=in_specs,
                  out_specs=out_specs, check_rep=False),
        donate_argnums=donate, keep_unused=True)
    _CACHE["runner"] = (sharded, in_names, out_names, out_avals, zero_shapes)
    return _CACHE["runner"]


def _run(in_maps):
    import jax
    import hashlib
    import jax
    from jax.sharding import NamedSharding, PartitionSpec
    sharded, in_names, out_names, out_avals, zero_shapes = _get_runner()
    PER_CORE = {"l1_lhsT", "l1_rhs", "nsq1"}
    sh = NamedSharding(_CACHE["mesh"], PartitionSpec("core"))

    # device-cache the (identical-across-calls) weight uploads
    wnames = [n for n in in_names if n not in PER_CORE]
    hsh = hashlib.md5()
    for n in wnames:
        hsh.update(np.asarray(in_maps[0][n]).tobytes())
    key = hsh.hexdigest()
    if _CACHE.get("dev_w_key") != key:
        _CACHE["dev_w"] = {
            n: jax.device_put(
                np.concatenate([np.asarray(in_maps[c][n])
                                for c in range(NCORES)], axis=0), sh)
            for n in wnames}
        _CACHE["dev_w_key"] = key

    concat_in = [
        np.concatenate([np.asarray(in_maps[c][name]) for c in range(NCORES)],
                       axis=0)
        if name in PER_CORE else _CACHE["dev_w"][name]
        for name in in_names]
    # donate last call's (fully-overwritten) outputs as this call's output
    # buffers; falls back to fresh zeros on the first call
    donbufs = _CACHE.pop("prev_outs", None)
    if donbufs is None:
        donbufs = [np.zeros((NCORES * s[0], *s[1:]), d)
                   for (s, d) in zero_shapes]
    try:
        out_arrs = jax.block_until_ready(sharded(*concat_in, *donbufs))
    except Exception:
        donbufs = [np.zeros((NCORES * s[0], *s[1:]), d)
                   for (s, d) in zero_shapes]
        out_arrs = jax.block_until_ready(sharded(*concat_in, *donbufs))
    results = []
    for c in range(NCORES):
        results.append({
            name: np.asarray(out_arrs[i]).reshape(
                NCORES, *out_avals[i].shape)[c]
            for i, name in enumerate(out_names)})
    _CACHE["prev_outs"] = list(out_arrs)
    return results


def kernel(xyz, params):
    xyz = _np(xyz)                                # [4, 4096, 3]
    wio = _weight_inputs(params)

    in_maps = []
    for core in range(NCORES):
        b, h = core // 2, core % 2
        cloud = xyz[b]                            # [4096, 3]
        own = cloud[h * HALF:(h + 1) * HALF]
        sq_full = (cloud * cloud).sum(-1)         # [4096]
        sq_own = sq_full[h * HALF:(h + 1) * HALF]
        im = dict(wio)
        im["l1_lhsT"] = np.concatenate(
            [own.T, np.ones((1, HALF), np.float32)], 0)
        im["l1_rhs"] = np.concatenate(
            [cloud.T, -0.5 * sq_full[None, :]], 0).astype(np.float32)
        im["nsq1"] = np.ascontiguousarray(
            (-sq_own).reshape(NT_OWN, 128).T)
        in_maps.append(im)

    import time
    t0 = time.perf_counter()
    results = _run(in_maps)
    _CACHE["last_exec_s"] = time.perf_counter() - t0
    out = np.zeros((B, N, NCLS), np.float32)
    for core in range(NCORES):
        b, h = core // 2, core % 2
        out[b, h * HALF:(h + 1) * HALF] = \
            results[core]["logits"].astype(np.float32)
    _CACHE["last_results"] = results
    return out
